# revision 13
# baseline (speedup 1.0000x reference)
"""MHA + RoPE fused kernel for Trainium2, sharded tensor-parallel over heads
across 8 NeuronCores.

Problem (hardcoded): B=4, S=2048, E=1024, H=16 heads, D=64.
  xq = x @ wq.T ; xk = x @ wk.T ; xv = x @ wv.T          [B,S,H,D]
  RoPE(xq, xk) with angles dt[b,s] * inv_freq[r]
  scores = softmax(xq @ xk.T / sqrt(D) + mask)            per (b, head)
  out = (scores @ xv) reshaped to [B,S,E]; y = out @ wo.T + bo
Each core owns 2 heads (128 q/k/v channels) and the matching 128 rows of
wo.T; it computes a full partial y (row-parallel output projection) and the
host sums the 8 bf16 partials.

Schedule: attention phase-2 runs in 16 blocks of (batch b, 512-token query
chunk iq).  Inside a block the 16 key-tile loop is software-pipelined: the
PE computes scores two key-tiles ahead of the ScalarE exp, and attn@V
trails right behind its exp, so the ScalarE (the global bottleneck at ~290us
of exp work) never starves and the PE never head-of-line blocks.  QKV
projections of the next batch and output projections of finished chunks are
chopped into ~1-2us "filler" units and woven between key-tiles with
deadline tags, keeping the PE continuously busy so the HAM clock gate stays
at K=8/8 (the baseline lost ~170us to cold-clock matmuls).

PSUM budget (8 banks): 2x scores [128,1024]f32 (4) + 2x attn-out [65,512]
accumulators (2) + 2x filler tiles [128,512] (2).
"""

import os
import sys

sys.path.insert(0, "/opt/trn_rl_repo")

import numpy as np
import ml_dtypes

DEBUG_DUMP = bool(os.environ.get("KERNEL_DEBUG_DUMP"))

import concourse.bass as bass
from concourse import bacc
import concourse.tile as tile
from concourse import mybir
from concourse.bass_utils import run_bass_kernel_spmd

F32 = mybir.dt.float32
BF16 = mybir.dt.bfloat16

B, S, E, H, D = 4, 2048, 1024, 16, 64
T = B * S                      # 8192 flattened tokens
NCORES = 8
HPC = H // NCORES              # 2 heads per core
CPC = HPC * D                  # 128 channels per core
NCHUNK = T // 512              # 16 token chunks
KT = E // 128                  # 8 contraction tiles
THETA = 10000.0
NEG_INF = -1e30

_prog_cache = {}


class _Unit:
    """One filler work unit: emit() puts ~0.5-2us of PE work (plus its DVE/
    DMA tail) into the instruction streams. due = (global tick) by which it
    must be emitted for correctness; budget pops usually emit it earlier."""

    __slots__ = ("emit", "due")

    def __init__(self, emit, due):
        self.emit = emit
        self.due = due


def _build_program(use_mask: bool):
    nc = bacc.Bacc()

    xT_d = nc.dram_tensor("xT", [E, T], BF16, kind="ExternalInput")
    cc_d = nc.dram_tensor("cc", [128, T], F32, kind="ExternalInput")
    ss_d = nc.dram_tensor("ss", [128, T], F32, kind="ExternalInput")
    wq_d = nc.dram_tensor("wqT", [E, CPC], BF16, kind="ExternalInput")
    wk_d = nc.dram_tensor("wkT", [E, CPC], BF16, kind="ExternalInput")
    wv_d = nc.dram_tensor("wvT", [E, CPC], BF16, kind="ExternalInput")
    wo_d = nc.dram_tensor("woT", [CPC, E], BF16, kind="ExternalInput")
    mb_d = None
    if use_mask:
        mb_d = nc.dram_tensor("mb", [128, B * 16], F32, kind="ExternalInput")
    y_d = nc.dram_tensor("yT", [E, T], BF16, kind="ExternalOutput")
    dbg_sc = dbg_pt = dbg_ocp = None
    if DEBUG_DUMP:
        dbg_sc = nc.dram_tensor("dbg_sc", [128, 16, 1024], F32,
                                kind="ExternalOutput")
        dbg_pt = nc.dram_tensor("dbg_pt", [128, 16, 1024], BF16,
                                kind="ExternalOutput")
        dbg_ocp = nc.dram_tensor("dbg_ocp", [130, 16, 512], F32,
                                 kind="ExternalOutput")

    xT_r = xT_d.rearrange("(k p) t -> p k t", p=128)
    wq_r = wq_d.rearrange("(k p) c -> p k c", p=128)
    wk_r = wk_d.rearrange("(k p) c -> p k c", p=128)
    wv_r = wv_d.rearrange("(k p) c -> p k c", p=128)
    wo_r = wo_d.rearrange("p (k c) -> p k c", c=128)

    with tile.TileContext(nc) as tc:
        with (
            tc.tile_pool(name="consts", bufs=1) as consts,
            tc.tile_pool(name="big", bufs=1) as big,
            tc.tile_pool(name="ph1", bufs=4) as ph1,
            tc.tile_pool(name="rope", bufs=2) as rope,
            tc.tile_pool(name="pt", bufs=4) as ptp,
            tc.tile_pool(name="norm", bufs=2) as norm,
            tc.tile_pool(name="ph3", bufs=3) as ph3,
            tc.tile_pool(name="psS", bufs=2, space="PSUM") as psS,
            tc.tile_pool(name="psA", bufs=2, space="PSUM") as psA,
            tc.tile_pool(name="psF", bufs=2, space="PSUM") as psF,
        ):
            # ---- constants ----
            wq_sb = consts.tile([128, KT, CPC], BF16)
            wk_sb = consts.tile([128, KT, CPC], BF16)
            wv_sb = consts.tile([128, KT, CPC], BF16)
            wo_sb = consts.tile([128, KT, 128], BF16)
            nc.sync.dma_start(wq_sb, wq_r)
            nc.sync.dma_start(wk_sb, wk_r)
            nc.sync.dma_start(wv_sb, wv_r)
            nc.sync.dma_start(wo_sb, wo_r)
            mb_sb = None
            if use_mask:
                mb_sb = consts.tile([128, B * 16], F32)
                nc.sync.dma_start(mb_sb, mb_d[:, :])
            ones65 = consts.tile([65, 64], F32)
            nc.vector.memset(ones65, 1.0)

            # ---- persistent activations ----
            qT_sb = big.tile([128, NCHUNK, 512], BF16)
            kT_sb = big.tile([128, NCHUNK, 512], BF16)
            # vAB[:, ti, 0:65] = head-A v dims 0-63 + ones col 64;
            # vAB[:, ti, 65:130] = head-B v dims + ones col 129
            vAB_sb = big.tile([128, T // 128, 130], BF16)
            attnT_sb = big.tile([128, NCHUNK, 512], BF16)
            nc.vector.memset(vAB_sb[:, :, 64], 1.0)
            nc.vector.memset(vAB_sb[:, :, 129], 1.0)

            # ---------- phase-1 units (QKV projection of one 512-tok chunk) --
            def load_x(ch):
                xsb = ph1.tile([128, KT, 512], BF16, tag="xsb")
                for k in range(KT):   # one DMA per k-tile -> spread queues
                    nc.sync.dma_start(xsb[:, k, :],
                                      xT_r[:, k, ch * 512:(ch + 1) * 512])
                cc_sb = ph1.tile([128, 512], F32, tag="cc")
                ss_sb = ph1.tile([128, 512], F32, tag="ss")
                for h0 in (0, 256):
                    nc.sync.dma_start(cc_sb[:, h0:h0 + 256],
                                      cc_d[:, ch * 512 + h0:ch * 512 + h0 + 256])
                    nc.sync.dma_start(ss_sb[:, h0:h0 + 256],
                                      ss_d[:, ch * 512 + h0:ch * 512 + h0 + 256])
                return xsb, cc_sb, ss_sb

            def qk_proj(ch, tiles, w_sb, dstT):
                """8 accumulating matmuls + RoPE -> qT/kT chunk."""
                xsb, cc_sb, ss_sb = tiles
                ps = psF.tile([128, 512], F32, tag="f", name="ps_qk")
                for k in range(KT):
                    nc.tensor.matmul(ps, w_sb[:, k, :], xsb[:, k, :],
                                     start=(k == 0), stop=(k == KT - 1))
                t1 = rope.tile([128, 512], F32, tag="t1")
                t2 = rope.tile([128, 512], F32, tag="t2")
                t2sw = rope.tile([128, 512], F32, tag="t2sw")
                nc.vector.tensor_tensor(t1, ps, cc_sb, mybir.AluOpType.mult)
                nc.vector.tensor_tensor(t2, ps, ss_sb, mybir.AluOpType.mult)
                for b0 in (0, 64):
                    nc.sync.dma_start(t2sw[b0:b0 + 32], t2[b0 + 32:b0 + 64])
                    nc.sync.dma_start(t2sw[b0 + 32:b0 + 64], t2[b0:b0 + 32])
                nc.vector.tensor_tensor(dstT[:, ch, :], t1, t2sw,
                                        mybir.AluOpType.add)

            def v_proj(ch, tiles, half):
                """V for token blocks 2*half, 2*half+1 of chunk ch."""
                xsb, _, _ = tiles
                for tt in (2 * half, 2 * half + 1):
                    psv = psF.tile([128, 128], F32, tag="f", name="psv")
                    for k in range(KT):
                        nc.tensor.matmul(psv, xsb[:, k, tt * 128:(tt + 1) * 128],
                                         wv_sb[:, k, :],
                                         start=(k == 0), stop=(k == KT - 1))
                    ti = ch * 4 + tt
                    # one strided copy fills both heads' v slices
                    dst = vAB_sb[:, ti, :].rearrange(
                        "p (g c) -> p g c", g=2)[:, :, 0:64]
                    src = psv[:, :].rearrange("p (g c) -> p g c", g=2)
                    nc.vector.tensor_copy(dst, src)

            # ---------- phase-3 unit (output projection of one chunk) -------
            def out_proj(ch, cck4):
                """4 of the 8 wo k-tiles for chunk ch."""
                for cck in range(cck4 * 4, cck4 * 4 + 4):
                    psy = psF.tile([128, 512], F32, tag="f", name="psy")
                    nc.tensor.matmul(psy, wo_sb[:, cck, :], attnT_sb[:, ch, :],
                                     start=True, stop=True)
                    ysb = ph3.tile([128, 512], BF16, tag="ysb")
                    nc.vector.tensor_copy(ysb, psy)
                    nc.sync.dma_start(
                        y_d[cck * 128:(cck + 1) * 128,
                            ch * 512:(ch + 1) * 512], ysb)

            # ---------- filler queue (kept sorted by due, FIFO on ties) ----
            queue = []
            _seq = [0]

            def enqueue(emit, due):
                import bisect
                _seq[0] += 1
                bisect.insort(queue, (due, _seq[0], _Unit(emit, due)))

            def pop_due(tick):
                while queue and queue[0][0] <= tick:
                    queue.pop(0)[2].emit()

            def pop_budget():
                if queue:
                    queue.pop(0)[2].emit()

            def enqueue_phase1(bn):
                """QKV units for all 4 chunks of batch bn.  kT / vA / vB / xsb
                feed matmul STATIONARY operands, and the PE's LDWEIGHTS
                pull-ahead reorder (64-deep window) does NOT respect the sems
                that gate the paired matmul — so all of load/k/v must be
                emitted >=2 blocks (>=64 PE instructions) before the first
                consuming block (bn, 0).  q feeds only MOVING operands (sem-
                gated properly), so q units may land as late as their block."""
                g0 = bn * 4 * 16        # tick of block (bn, 0), 16 ticks/block
                kv_due = g0 - 32
                tiles = {}
                # All 4 loads first, then all k, then all v, then all q:
                # v_proj consumes xsb as a matmul STATIONARY, and the PE's
                # LDWEIGHTS pull-ahead can read it up to ~64 instructions
                # early -- the k units (32 matmuls) in between guarantee the
                # DMA has landed before any v LDWEIGHTS can issue.
                for c4 in range(4):
                    ch = bn * 4 + c4

                    def em_load(ch=ch):
                        tiles[ch] = load_x(ch)
                    enqueue(em_load, kv_due)
                for c4 in range(4):
                    ch = bn * 4 + c4
                    enqueue(
                        lambda ch=ch: qk_proj(ch, tiles[ch], wk_sb, kT_sb),
                        kv_due + 1)
                for c4 in range(4):
                    ch = bn * 4 + c4
                    for half in (0, 1):
                        enqueue(
                            lambda ch=ch, half=half: v_proj(ch, tiles[ch], half),
                            kv_due + 2)
                # q last: frees this batch's xsb slots before the NEXT
                # batch's loads (sorted-queue order); q feeds only moving
                # operands, so closeness to its consumers is safe.
                for c4 in range(4):
                    ch = bn * 4 + c4
                    enqueue(
                        lambda ch=ch: qk_proj(ch, tiles[ch], wq_sb, qT_sb),
                        kv_due + 3)

            def enqueue_phase3(ch):
                for cck4 in (0, 1):
                    enqueue(lambda ch=ch, cck4=cck4: out_proj(ch, cck4), 1 << 30)

            # ---------- phase-2 block --------------------------------------
            def phase2_block(b, iq):
                g = (b * 4 + iq) * 16
                ch_i = b * 4 + iq
                posA = psA.tile([65, 512], F32, tag="pos", name="posA")
                posB = psA.tile([65, 512], F32, tag="pos", name="posB")
                pss = {}

                def scores(jb):
                    t = psS.tile([128, 1024], F32, tag="sc", name=f"sc{jb % 2}")
                    pss[jb] = t
                    ch_j = b * 4 + jb // 4
                    off_j = (jb % 4) * 128
                    for hh, b0 in ((0, 0), (1, 64)):
                        nc.tensor.matmul(
                            t[:, hh * 512:(hh + 1) * 512],
                            kT_sb[b0:b0 + 64, ch_j, off_j:off_j + 128],
                            qT_sb[b0:b0 + 64, ch_i, :],
                            start=True, stop=True, tile_position=(b0, 0))

                pop_due(g - 1)
                scores(0)
                scores(1)
                for jb in range(16):
                    pop_due(g + jb)
                    if jb == 1 and pending_fin:
                        # must emit before any pop_budget() can emit an
                        # out_proj that reads the attnT these fill in
                        for f in pending_fin:
                            f()
                        pending_fin.clear()
                    bias = (mb_sb[:, b * 16 + jb:b * 16 + jb + 1]
                            if use_mask else 0.0)
                    pT = ptp.tile([128, 1024], BF16, tag="pT", name="pT")
                    ptile = pss.pop(jb)
                    nc.scalar.activation(pT, ptile,
                                         mybir.ActivationFunctionType.Exp,
                                         bias=bias, scale=0.125)
                    if DEBUG_DUMP and b == 0 and iq == 0:
                        dsc = norm.tile([128, 1024], F32, tag="dbgsc")
                        nc.vector.tensor_copy(dsc, ptile)
                        nc.sync.dma_start(dbg_sc[:, jb, :], dsc)
                        nc.sync.dma_start(dbg_pt[:, jb, :], pT)
                    if jb + 2 < 16:
                        scores(jb + 2)
                    if jb % 2 == 1:
                        pop_budget()
                    for hh, pos in ((0, posA), (1, posB)):
                        nc.tensor.matmul(
                            pos,
                            vAB_sb[:, b * 16 + jb, hh * 65:(hh + 1) * 65],
                            pT[:, hh * 512:(hh + 1) * 512],
                            start=(jb == 0), stop=(jb == 15))
                # normalize: out = attn_out / colsum, written transposed
                # bf16.  ocp+csrow copies happen now (frees the pos banks);
                # the reciprocal, PE broadcast and multiply are deferred
                # into the next block so the PE never head-of-line waits on
                # the DVE.
                rows = []
                for hh, pos in ((0, posA), (1, posB)):
                    ocp = norm.tile([65, 512], F32, tag="ocp")
                    nc.vector.tensor_copy(ocp, pos)
                    # reciprocal_approx_fast needs a base-partition-0 input
                    # tile; feeding it a [64:65] slice mis-executes
                    csrow = norm.tile([1, 512], F32, tag="csrow")
                    nc.vector.tensor_copy(csrow, ocp[64:65, :])
                    rows.append((hh, ocp, csrow))

                def finish_norm():
                    for hh, ocp, csrow in rows:
                        csrec = norm.tile([1, 512], F32, tag="csrec")
                        nc.vector.reciprocal_approx_fast(out=csrec, in_=csrow)
                        # broadcast recip to 64 partitions with a K=1 matmul
                        # (ones stationary x csrec moving)
                        csrep = psF.tile([64, 512], F32, tag="f",
                                         name="csrep")
                        nc.tensor.matmul(csrep, ones65[0:1, :], csrec,
                                         start=True, stop=True)
                        if DEBUG_DUMP:
                            nc.sync.dma_start(
                                dbg_ocp[hh * 65:hh * 65 + 65, ch_i, :], ocp)
                        nc.vector.tensor_tensor(
                            attnT_sb[hh * 64:hh * 64 + 64, ch_i, :],
                            ocp[0:64, :], csrep, mybir.AluOpType.mult)
                pending_fin.append(finish_norm)

            # ---------- program --------------------------------------------
            # prologue: all of batch 0's load/k/v inline (stationary-feeding
            # data must be far upstream of its consumers, see enqueue_phase1)
            t0 = {ch: load_x(ch) for ch in range(4)}
            for ch in range(4):
                qk_proj(ch, t0[ch], wk_sb, kT_sb)
            for ch in range(4):
                v_proj(ch, t0[ch], 0)
                v_proj(ch, t0[ch], 1)
            for ch in range(4):
                qk_proj(ch, t0[ch], wq_sb, qT_sb)

            pending_fin = []
            for b in range(B):
                if b + 1 < B:
                    enqueue_phase1(b + 1)
                for iq in range(4):
                    phase2_block(b, iq)
                    enqueue_phase3(b * 4 + iq)
            for f in pending_fin:
                f()
            pending_fin.clear()
            while queue:
                queue.pop(0)[2].emit()

    return nc


def _host_prep(x, key_padding_mask, dt, wq, wk, wv, wo):
    """Shared + per-core input arrays (all numpy)."""
    xT = np.ascontiguousarray(x.reshape(T, E).T).astype(ml_dtypes.bfloat16)

    # RoPE trig tables, rows [c;c;c;c] and [s;-s;s;-s] over 32-row blocks
    inv_freq = (1.0 / (THETA ** (np.arange(0, D, 2, dtype=np.float32) / D)))
    ang = dt.reshape(T).astype(np.float32)[None, :] * inv_freq[:, None]  # [32, T]
    cos = np.cos(ang).astype(np.float32)
    sin = np.sin(ang).astype(np.float32)
    cc = np.concatenate([cos, cos, cos, cos], axis=0)
    ssm = np.concatenate([sin, -sin, sin, -sin], axis=0)

    use_mask = bool(key_padding_mask.any())
    mb = None
    if use_mask:
        bias = np.where(key_padding_mask.reshape(T), NEG_INF, 0.0).astype(np.float32)
        # [128 j-in-block, B*16 block index]
        mb = np.ascontiguousarray(bias.reshape(B * 16, 128).T)

    # per-head channel permutation: [2r] then [2r+1] -> [r | 32+r]
    perm1 = np.concatenate([np.arange(0, D, 2), np.arange(1, D, 2)])

    per_core = []
    for c in range(NCORES):
        rows = []
        for h in range(c * HPC, (c + 1) * HPC):
            rows.append(h * D + perm1)
        rows = np.concatenate(rows)                      # permuted q/k rows
        rows_v = np.arange(c * CPC, (c + 1) * CPC)       # natural v rows
        # note: the 1/sqrt(D)=0.125 score scale is applied as the exp
        # activation's scale argument on device, not here
        wqT = np.ascontiguousarray(wq[rows].T).astype(ml_dtypes.bfloat16)
        wkT = np.ascontiguousarray(wk[rows].T).astype(ml_dtypes.bfloat16)
        wvT = np.ascontiguousarray(wv[rows_v].T).astype(ml_dtypes.bfloat16)
        woT = np.ascontiguousarray(wo[:, rows_v].T).astype(ml_dtypes.bfloat16)
        m = {"xT": xT, "cc": cc, "ss": ssm,
             "wqT": wqT, "wkT": wkT, "wvT": wvT, "woT": woT}
        if use_mask:
            m["mb"] = mb
        per_core.append(m)
    return per_core, use_mask


def kernel(x, key_padding_mask, dt, wq, wk, wv, wo, bo, _return_results=False):
    x = np.asarray(x, dtype=np.float32)
    key_padding_mask = np.asarray(key_padding_mask)
    dt = np.asarray(dt, dtype=np.float32)
    wq = np.asarray(wq, dtype=np.float32)
    wk = np.asarray(wk, dtype=np.float32)
    wv = np.asarray(wv, dtype=np.float32)
    wo = np.asarray(wo, dtype=np.float32)
    bo = np.asarray(bo, dtype=np.float32)

    in_maps, use_mask = _host_prep(x, key_padding_mask, dt, wq, wk, wv, wo)

    key = use_mask
    if key not in _prog_cache:
        prog = _build_program(use_mask)
        prog.finalize()
        _prog_cache[key] = prog
    nc = _prog_cache[key]

    res = run_bass_kernel_spmd(nc, in_maps, list(range(NCORES)))

    y = np.zeros((E, T), dtype=np.float32)
    for r in res.results:
        y += r["yT"].astype(np.float32)
    out = (y.T + bo[None, :]).reshape(B, S, E).astype(np.float32)
    if _return_results:
        return out, res
    return out



# revision 18
# speedup vs baseline: 1.2677x; 1.2677x over previous
"""MHA + RoPE fused kernel for Trainium2, sharded tensor-parallel over heads
across 8 NeuronCores.

Problem (hardcoded): B=4, S=2048, E=1024, H=16 heads, D=64.
  xq = x @ wq.T ; xk = x @ wk.T ; xv = x @ wv.T          [B,S,H,D]
  RoPE(xq, xk) with angles dt[b,s] * inv_freq[r]
  scores = softmax(xq @ xk.T / sqrt(D) + mask)            per (b, head)
  out = (scores @ xv) reshaped to [B,S,E]; y = out @ wo.T + bo
Each core owns 2 heads (128 q/k/v channels) and the matching 128 rows of
wo.T; it computes a full partial y (row-parallel output projection) and the
host sums the 8 bf16 partials.

Schedule: attention phase-2 runs in 16 blocks of (batch b, 512-token query
chunk iq).  Inside a block the 16 key-tile loop is software-pipelined: the
PE computes scores two key-tiles ahead of the ScalarE exp, and attn@V
trails right behind its exp, so the ScalarE (the global bottleneck at ~290us
of exp work) never starves and the PE never head-of-line blocks.  QKV
projections of the next batch and output projections of finished chunks are
chopped into ~1-2us "filler" units and woven between key-tiles with
deadline tags, keeping the PE continuously busy so the HAM clock gate stays
at K=8/8 (the baseline lost ~170us to cold-clock matmuls).

PSUM budget (8 banks): 2x scores [128,1024]f32 (4) + 2x attn-out [65,512]
accumulators (2) + 2x filler tiles [128,512] (2).
"""

import os
import sys

sys.path.insert(0, "/opt/trn_rl_repo")

import numpy as np
import ml_dtypes

DEBUG_DUMP = bool(os.environ.get("KERNEL_DEBUG_DUMP"))

import concourse.bass as bass
from concourse import bacc
import concourse.tile as tile
from concourse import mybir
from concourse.bass_utils import run_bass_kernel_spmd

F32 = mybir.dt.float32
BF16 = mybir.dt.bfloat16

B, S, E, H, D = 4, 2048, 1024, 16, 64
T = B * S                      # 8192 flattened tokens
NCORES = 8
HPC = H // NCORES              # 2 heads per core
CPC = HPC * D                  # 128 channels per core
NCHUNK = T // 512              # 16 token chunks
KT = E // 128                  # 8 contraction tiles
THETA = 10000.0
NEG_INF = -1e30

_prog_cache = {}


class _Unit:
    """One filler work unit: emit() puts ~0.5-2us of PE work (plus its DVE/
    DMA tail) into the instruction streams. due = (global tick) by which it
    must be emitted for correctness; budget pops usually emit it earlier."""

    __slots__ = ("emit", "due")

    def __init__(self, emit, due):
        self.emit = emit
        self.due = due


def _build_program(use_mask: bool):
    nc = bacc.Bacc()

    xT_d = nc.dram_tensor("xT", [E, T], BF16, kind="ExternalInput")
    cc_d = nc.dram_tensor("cc", [128, T], F32, kind="ExternalInput")
    ss_d = nc.dram_tensor("ss", [128, T], F32, kind="ExternalInput")
    wq_d = nc.dram_tensor("wqT", [E, CPC], BF16, kind="ExternalInput")
    wk_d = nc.dram_tensor("wkT", [E, CPC], BF16, kind="ExternalInput")
    wv_d = nc.dram_tensor("wvT", [E, CPC], BF16, kind="ExternalInput")
    wo_d = nc.dram_tensor("woT", [CPC, E], BF16, kind="ExternalInput")
    mb_d = None
    if use_mask:
        mb_d = nc.dram_tensor("mb", [128, B * 16], F32, kind="ExternalInput")
    y_d = nc.dram_tensor("yT", [E, T], BF16, kind="ExternalOutput")
    dbg_sc = dbg_pt = dbg_ocp = None
    if DEBUG_DUMP:
        dbg_sc = nc.dram_tensor("dbg_sc", [128, 16, 1024], F32,
                                kind="ExternalOutput")
        dbg_pt = nc.dram_tensor("dbg_pt", [128, 16, 1024], BF16,
                                kind="ExternalOutput")
        dbg_ocp = nc.dram_tensor("dbg_ocp", [130, 16, 512], F32,
                                 kind="ExternalOutput")

    xT_r = xT_d.rearrange("(k p) t -> p k t", p=128)
    wq_r = wq_d.rearrange("(k p) c -> p k c", p=128)
    wk_r = wk_d.rearrange("(k p) c -> p k c", p=128)
    wv_r = wv_d.rearrange("(k p) c -> p k c", p=128)
    wo_r = wo_d.rearrange("p (k c) -> p k c", c=128)

    with tile.TileContext(nc) as tc:
        with (
            tc.tile_pool(name="consts", bufs=1) as consts,
            tc.tile_pool(name="big", bufs=1) as big,
            tc.tile_pool(name="ph1", bufs=4) as ph1,
            tc.tile_pool(name="rope", bufs=2) as rope,
            tc.tile_pool(name="pt", bufs=4) as ptp,
            tc.tile_pool(name="norm", bufs=2) as norm,
            tc.tile_pool(name="ph3", bufs=3) as ph3,
            tc.tile_pool(name="psS", bufs=2, space="PSUM") as psS,
            tc.tile_pool(name="psA", bufs=2, space="PSUM") as psA,
            tc.tile_pool(name="psF", bufs=2, space="PSUM") as psF,
        ):
            # ---- constants ----
            wq_sb = consts.tile([128, KT, CPC], BF16)
            wk_sb = consts.tile([128, KT, CPC], BF16)
            wv_sb = consts.tile([128, KT, CPC], BF16)
            wo_sb = consts.tile([128, KT, 128], BF16)
            nc.sync.dma_start(wq_sb, wq_r)
            nc.sync.dma_start(wk_sb, wk_r)
            nc.sync.dma_start(wv_sb, wv_r)
            nc.sync.dma_start(wo_sb, wo_r)
            mb_sb = None
            if use_mask:
                mb_sb = consts.tile([128, B * 16], F32)
                nc.sync.dma_start(mb_sb, mb_d[:, :])
            # bf16: an fp32 stationary would put the broadcast matmul in
            # fp32 LOW/HIGH mode (two ~750ns passes instead of one 213ns)
            ones65 = consts.tile([65, 64], BF16)
            nc.vector.memset(ones65, 1.0)

            # ---- persistent activations ----
            qT_sb = big.tile([128, NCHUNK, 512], BF16)
            kT_sb = big.tile([128, NCHUNK, 512], BF16)
            # vAB[:, ti, 0:65] = head-A v dims 0-63 + ones col 64;
            # vAB[:, ti, 65:130] = head-B v dims + ones col 129
            vAB_sb = big.tile([128, T // 128, 130], BF16)
            attnT_sb = big.tile([128, NCHUNK, 512], BF16)
            nc.vector.memset(vAB_sb[:, :, 64], 1.0)
            nc.vector.memset(vAB_sb[:, :, 129], 1.0)

            # ---------- phase-1 units (QKV projection of one 512-tok chunk) --
            def load_x(ch):
                xsb = ph1.tile([128, KT, 512], BF16, tag="xsb")
                for k in range(KT):   # one DMA per k-tile -> spread queues
                    nc.sync.dma_start(xsb[:, k, :],
                                      xT_r[:, k, ch * 512:(ch + 1) * 512])
                cc_sb = ph1.tile([128, 512], F32, tag="cc")
                ss_sb = ph1.tile([128, 512], F32, tag="ss")
                for h0 in (0, 256):
                    nc.sync.dma_start(cc_sb[:, h0:h0 + 256],
                                      cc_d[:, ch * 512 + h0:ch * 512 + h0 + 256])
                    nc.sync.dma_start(ss_sb[:, h0:h0 + 256],
                                      ss_d[:, ch * 512 + h0:ch * 512 + h0 + 256])
                return xsb, cc_sb, ss_sb

            def qk_proj(ch, tiles, w_sb, dstT):
                """8 accumulating matmuls + RoPE -> qT/kT chunk."""
                xsb, cc_sb, ss_sb = tiles
                ps = psF.tile([128, 512], F32, tag="f", name="ps_qk")
                for k in range(KT):
                    nc.tensor.matmul(ps, w_sb[:, k, :], xsb[:, k, :],
                                     start=(k == 0), stop=(k == KT - 1))
                t1 = rope.tile([128, 512], F32, tag="t1")
                t2 = rope.tile([128, 512], F32, tag="t2")
                t2sw = rope.tile([128, 512], F32, tag="t2sw")
                nc.vector.tensor_tensor(t1, ps, cc_sb, mybir.AluOpType.mult)
                nc.vector.tensor_tensor(t2, ps, ss_sb, mybir.AluOpType.mult)
                for b0 in (0, 64):
                    nc.sync.dma_start(t2sw[b0:b0 + 32], t2[b0 + 32:b0 + 64])
                    nc.sync.dma_start(t2sw[b0 + 32:b0 + 64], t2[b0:b0 + 32])
                nc.vector.tensor_tensor(dstT[:, ch, :], t1, t2sw,
                                        mybir.AluOpType.add)

            def v_proj(ch, tiles, half):
                """V for token blocks 2*half, 2*half+1 of chunk ch."""
                xsb, _, _ = tiles
                for tt in (2 * half, 2 * half + 1):
                    psv = psF.tile([128, 128], F32, tag="f", name="psv")
                    for k in range(KT):
                        nc.tensor.matmul(psv, xsb[:, k, tt * 128:(tt + 1) * 128],
                                         wv_sb[:, k, :],
                                         start=(k == 0), stop=(k == KT - 1))
                    ti = ch * 4 + tt
                    # one strided copy fills both heads' v slices
                    dst = vAB_sb[:, ti, :].rearrange(
                        "p (g c) -> p g c", g=2)[:, :, 0:64]
                    src = psv[:, :].rearrange("p (g c) -> p g c", g=2)
                    nc.vector.tensor_copy(dst, src)

            # ---------- phase-3 unit (output projection of one chunk) -------
            def out_proj(ch, cck4):
                """4 of the 8 wo k-tiles for chunk ch."""
                for cck in range(cck4 * 4, cck4 * 4 + 4):
                    psy = psF.tile([128, 512], F32, tag="f", name="psy")
                    nc.tensor.matmul(psy, wo_sb[:, cck, :], attnT_sb[:, ch, :],
                                     start=True, stop=True)
                    ysb = ph3.tile([128, 512], BF16, tag="ysb")
                    nc.vector.tensor_copy(ysb, psy)
                    nc.sync.dma_start(
                        y_d[cck * 128:(cck + 1) * 128,
                            ch * 512:(ch + 1) * 512], ysb)

            # ---------- filler queue (kept sorted by due, FIFO on ties) ----
            queue = []
            _seq = [0]

            def enqueue(emit, due):
                import bisect
                _seq[0] += 1
                bisect.insort(queue, (due, _seq[0], _Unit(emit, due)))

            def pop_due(tick):
                while queue and queue[0][0] <= tick:
                    queue.pop(0)[2].emit()

            def pop_budget():
                if queue:
                    queue.pop(0)[2].emit()

            def enqueue_phase1(bn):
                """QKV units for all 4 chunks of batch bn.  kT / vA / vB / xsb
                feed matmul STATIONARY operands, and the PE's LDWEIGHTS
                pull-ahead reorder (64-deep window) does NOT respect the sems
                that gate the paired matmul — so all of load/k/v must be
                emitted >=2 blocks (>=64 PE instructions) before the first
                consuming block (bn, 0).  q feeds only MOVING operands (sem-
                gated properly), so q units may land as late as their block."""
                g0 = bn * 4 * 16        # tick of block (bn, 0), 16 ticks/block
                kv_due = g0 - 32
                tiles = {}
                # All 4 loads first, then all k, then all v, then all q:
                # v_proj consumes xsb as a matmul STATIONARY, and the PE's
                # LDWEIGHTS pull-ahead can read it up to ~64 instructions
                # early -- the k units (32 matmuls) in between guarantee the
                # DMA has landed before any v LDWEIGHTS can issue.
                for c4 in range(4):
                    ch = bn * 4 + c4

                    def em_load(ch=ch):
                        tiles[ch] = load_x(ch)
                    enqueue(em_load, kv_due)
                for c4 in range(4):
                    ch = bn * 4 + c4
                    enqueue(
                        lambda ch=ch: qk_proj(ch, tiles[ch], wk_sb, kT_sb),
                        kv_due + 1)
                for c4 in range(4):
                    ch = bn * 4 + c4
                    for half in (0, 1):
                        enqueue(
                            lambda ch=ch, half=half: v_proj(ch, tiles[ch], half),
                            kv_due + 2)
                # q last: frees this batch's xsb slots before the NEXT
                # batch's loads (sorted-queue order); q feeds only moving
                # operands, so closeness to its consumers is safe.
                for c4 in range(4):
                    ch = bn * 4 + c4
                    enqueue(
                        lambda ch=ch: qk_proj(ch, tiles[ch], wq_sb, qT_sb),
                        kv_due + 3)

            def enqueue_phase3(ch):
                for cck4 in (0, 1):
                    enqueue(lambda ch=ch, cck4=cck4: out_proj(ch, cck4), 1 << 30)

            # ---------- phase-2 block --------------------------------------
            def phase2_block(b, iq):
                g = (b * 4 + iq) * 16
                ch_i = b * 4 + iq
                posA = psA.tile([65, 512], F32, tag="pos", name="posA")
                posB = psA.tile([65, 512], F32, tag="pos", name="posB")
                pss = {}

                def scores(jb):
                    t = psS.tile([128, 1024], F32, tag="sc", name=f"sc{jb % 2}")
                    pss[jb] = t
                    ch_j = b * 4 + jb // 4
                    off_j = (jb % 4) * 128
                    for hh, b0 in ((0, 0), (1, 64)):
                        nc.tensor.matmul(
                            t[:, hh * 512:(hh + 1) * 512],
                            kT_sb[b0:b0 + 64, ch_j, off_j:off_j + 128],
                            qT_sb[b0:b0 + 64, ch_i, :],
                            start=True, stop=True, tile_position=(b0, 0))

                pop_due(g - 1)
                scores(0)
                scores(1)
                for jb in range(16):
                    pop_due(g + jb)
                    if jb == 1 and pending_fin:
                        # must emit before any pop_budget() can emit an
                        # out_proj that reads the attnT these fill in
                        for f in pending_fin:
                            f()
                        pending_fin.clear()
                    bias = (mb_sb[:, b * 16 + jb:b * 16 + jb + 1]
                            if use_mask else 0.0)
                    pT = ptp.tile([128, 1024], BF16, tag="pT", name="pT")
                    ptile = pss.pop(jb)
                    nc.scalar.activation(pT, ptile,
                                         mybir.ActivationFunctionType.Exp,
                                         bias=bias, scale=0.125)
                    if DEBUG_DUMP and b == 0 and iq == 0:
                        dsc = norm.tile([128, 1024], F32, tag="dbgsc")
                        nc.vector.tensor_copy(dsc, ptile)
                        nc.sync.dma_start(dbg_sc[:, jb, :], dsc)
                        nc.sync.dma_start(dbg_pt[:, jb, :], pT)
                    if jb + 2 < 16:
                        scores(jb + 2)
                    if jb % 2 == 1:
                        pop_budget()
                        if b == B - 1:
                            # no next-batch phase-1 exists; drain the
                            # out_proj backlog instead of leaving a tail
                            pop_budget()
                    for hh, pos in ((0, posA), (1, posB)):
                        nc.tensor.matmul(
                            pos,
                            vAB_sb[:, b * 16 + jb, hh * 65:(hh + 1) * 65],
                            pT[:, hh * 512:(hh + 1) * 512],
                            start=(jb == 0), stop=(jb == 15))
                # normalize: out = attn_out / colsum, written transposed
                # bf16.  ocp+csrow copies happen now (frees the pos banks);
                # the reciprocal, PE broadcast and multiply are deferred
                # into the next block so the PE never head-of-line waits on
                # the DVE.
                rows = []
                for hh, pos in ((0, posA), (1, posB)):
                    ocp = norm.tile([65, 512], F32, tag="ocp")
                    nc.vector.tensor_copy(ocp, pos)
                    # reciprocal_approx_fast needs a base-partition-0 input
                    # tile; feeding it a [64:65] slice mis-executes
                    csrow = norm.tile([1, 512], F32, tag="csrow")
                    nc.vector.tensor_copy(csrow, ocp[64:65, :])
                    rows.append((hh, ocp, csrow))

                def finish_norm():
                    for hh, ocp, csrow in rows:
                        csrec = norm.tile([1, 512], F32, tag="csrec")
                        nc.vector.reciprocal_approx_fast(out=csrec, in_=csrow)
                        # bf16 copy keeps the broadcast matmul out of fp32
                        # mode; the ~0.4% recip error scales whole columns
                        # uniformly and is negligible vs the 2e-2 budget
                        csrecb = norm.tile([1, 512], BF16, tag="csrecb")
                        nc.vector.tensor_copy(csrecb, csrec)
                        # broadcast recip to 64 partitions with a K=1 matmul
                        # (ones stationary x csrec moving)
                        csrep = psF.tile([64, 512], F32, tag="f",
                                         name="csrep")
                        nc.tensor.matmul(csrep, ones65[0:1, :], csrecb,
                                         start=True, stop=True)
                        if DEBUG_DUMP:
                            nc.sync.dma_start(
                                dbg_ocp[hh * 65:hh * 65 + 65, ch_i, :], ocp)
                        nc.vector.tensor_tensor(
                            attnT_sb[hh * 64:hh * 64 + 64, ch_i, :],
                            ocp[0:64, :], csrep, mybir.AluOpType.mult)
                if b == B - 1 and iq == 3:
                    finish_norm()   # last block: no next block to defer into
                else:
                    pending_fin.append(finish_norm)

            # ---------- program --------------------------------------------
            # prologue: all of batch 0's load/k/v inline (stationary-feeding
            # data must be far upstream of its consumers, see enqueue_phase1)
            t0 = {ch: load_x(ch) for ch in range(4)}
            for ch in range(4):
                qk_proj(ch, t0[ch], wk_sb, kT_sb)
            for ch in range(4):
                v_proj(ch, t0[ch], 0)
                v_proj(ch, t0[ch], 1)
            # only chunk 0's q is needed before block (0,0) starts (scores
            # consume qT as a sem-gated moving operand, so tight is safe);
            # q1-3 weave into the first blocks as filler
            qk_proj(0, t0[0], wq_sb, qT_sb)
            for ch in (1, 2, 3):
                enqueue(
                    lambda ch=ch: qk_proj(ch, t0[ch], wq_sb, qT_sb), ch - 1)

            pending_fin = []
            for b in range(B):
                if b + 1 < B:
                    enqueue_phase1(b + 1)
                for iq in range(4):
                    phase2_block(b, iq)
                    enqueue_phase3(b * 4 + iq)
            for f in pending_fin:
                f()
            pending_fin.clear()
            while queue:
                queue.pop(0)[2].emit()

    return nc


def _host_prep(x, key_padding_mask, dt, wq, wk, wv, wo):
    """Shared + per-core input arrays (all numpy)."""
    xT = np.ascontiguousarray(x.reshape(T, E).T).astype(ml_dtypes.bfloat16)

    # RoPE trig tables, rows [c;c;c;c] and [s;-s;s;-s] over 32-row blocks
    inv_freq = (1.0 / (THETA ** (np.arange(0, D, 2, dtype=np.float32) / D)))
    ang = dt.reshape(T).astype(np.float32)[None, :] * inv_freq[:, None]  # [32, T]
    cos = np.cos(ang).astype(np.float32)
    sin = np.sin(ang).astype(np.float32)
    cc = np.concatenate([cos, cos, cos, cos], axis=0)
    ssm = np.concatenate([sin, -sin, sin, -sin], axis=0)

    use_mask = bool(key_padding_mask.any())
    mb = None
    if use_mask:
        bias = np.where(key_padding_mask.reshape(T), NEG_INF, 0.0).astype(np.float32)
        # [128 j-in-block, B*16 block index]
        mb = np.ascontiguousarray(bias.reshape(B * 16, 128).T)

    # per-head channel permutation: [2r] then [2r+1] -> [r | 32+r]
    perm1 = np.concatenate([np.arange(0, D, 2), np.arange(1, D, 2)])

    per_core = []
    for c in range(NCORES):
        rows = []
        for h in range(c * HPC, (c + 1) * HPC):
            rows.append(h * D + perm1)
        rows = np.concatenate(rows)                      # permuted q/k rows
        rows_v = np.arange(c * CPC, (c + 1) * CPC)       # natural v rows
        # note: the 1/sqrt(D)=0.125 score scale is applied as the exp
        # activation's scale argument on device, not here
        wqT = np.ascontiguousarray(wq[rows].T).astype(ml_dtypes.bfloat16)
        wkT = np.ascontiguousarray(wk[rows].T).astype(ml_dtypes.bfloat16)
        wvT = np.ascontiguousarray(wv[rows_v].T).astype(ml_dtypes.bfloat16)
        woT = np.ascontiguousarray(wo[:, rows_v].T).astype(ml_dtypes.bfloat16)
        m = {"xT": xT, "cc": cc, "ss": ssm,
             "wqT": wqT, "wkT": wkT, "wvT": wvT, "woT": woT}
        if use_mask:
            m["mb"] = mb
        per_core.append(m)
    return per_core, use_mask


def kernel(x, key_padding_mask, dt, wq, wk, wv, wo, bo, _return_results=False):
    x = np.asarray(x, dtype=np.float32)
    key_padding_mask = np.asarray(key_padding_mask)
    dt = np.asarray(dt, dtype=np.float32)
    wq = np.asarray(wq, dtype=np.float32)
    wk = np.asarray(wk, dtype=np.float32)
    wv = np.asarray(wv, dtype=np.float32)
    wo = np.asarray(wo, dtype=np.float32)
    bo = np.asarray(bo, dtype=np.float32)

    in_maps, use_mask = _host_prep(x, key_padding_mask, dt, wq, wk, wv, wo)

    key = use_mask
    if key not in _prog_cache:
        prog = _build_program(use_mask)
        prog.finalize()
        _prog_cache[key] = prog
    nc = _prog_cache[key]

    res = run_bass_kernel_spmd(nc, in_maps, list(range(NCORES)))

    y = np.zeros((E, T), dtype=np.float32)
    for r in res.results:
        y += r["yT"].astype(np.float32)
    out = (y.T + bo[None, :]).reshape(B, S, E).astype(np.float32)
    if _return_results:
        return out, res
    return out



# revision 23
# speedup vs baseline: 1.2936x; 1.0205x over previous
"""MHA + RoPE fused kernel for Trainium2, sharded tensor-parallel over heads
across 8 NeuronCores.

Problem (hardcoded): B=4, S=2048, E=1024, H=16 heads, D=64.
  xq = x @ wq.T ; xk = x @ wk.T ; xv = x @ wv.T          [B,S,H,D]
  RoPE(xq, xk) with angles dt[b,s] * inv_freq[r]
  scores = softmax(xq @ xk.T / sqrt(D) + mask)            per (b, head)
  out = (scores @ xv) reshaped to [B,S,E]; y = out @ wo.T + bo
Each core owns 2 heads (128 q/k/v channels) and the matching 128 rows of
wo.T; it computes a full partial y (row-parallel output projection) and the
host sums the 8 bf16 partials.

Schedule: attention phase-2 runs in 16 blocks of (batch b, 512-token query
chunk iq).  Inside a block the 16 key-tile loop is software-pipelined: the
PE computes scores two key-tiles ahead of the ScalarE exp, and attn@V
trails right behind its exp, so the ScalarE (~286us of exp work) never
starves and the PE never head-of-line blocks.  QKV projections of the next
batch and output projections of finished chunks are chopped into ~1-2us
"filler" units and woven between key-tiles with deadline tags, keeping the
PE continuously busy so the HAM clock gate stays at K=8/8.

vs. the 524us baseline (now ~460us):
 - softmax normalize is split: the pos->SBUF copies run at block end, but
   reciprocal + K=1-broadcast-matmul + multiply are deferred into the next
   block so the PE never waits on the DVE;
 - the broadcast matmul runs in bf16 (an fp32 ones vector put it in fp32
   LOW/HIGH mode: two ~750ns passes instead of one ~230ns - 45us of PE);
 - vA/vB live interleaved in one [128, ti, 130] tile so each v-projection
   drains with one strided copy instead of two;
 - only chunk 0's q-projection precedes block (0,0); q1-3 weave into the
   first block as filler (shorter prologue);
 - DMAs are spread across the Sync HWDGE, Scalar HWDGE (prologue only) and
   GpSimd SWDGE queues; y-writeback rides GpSimd so loads never queue
   behind it;
 - the last batch drains the out_proj backlog with doubled budget pops.

fp8 (DoubleRow) was evaluated and rejected: quantizing any single tensor
of the attention path (pT, V, attnT, or wo) to e4m3 already exceeds the
2e-2 max-rel-error budget (pT alone: 0.16).

PSUM budget (8 banks): 2x scores [128,1024]f32 (4) + 2x attn-out [65,512]
accumulators (2) + 2x filler tiles [128,512] (2).
"""

import os
import sys

sys.path.insert(0, "/opt/trn_rl_repo")

import numpy as np
import ml_dtypes

DEBUG_DUMP = bool(os.environ.get("KERNEL_DEBUG_DUMP"))

import concourse.bass as bass
from concourse import bacc
import concourse.tile as tile
from concourse import mybir
from concourse.bass_utils import run_bass_kernel_spmd

F32 = mybir.dt.float32
BF16 = mybir.dt.bfloat16

B, S, E, H, D = 4, 2048, 1024, 16, 64
T = B * S                      # 8192 flattened tokens
NCORES = 8
HPC = H // NCORES              # 2 heads per core
CPC = HPC * D                  # 128 channels per core
NCHUNK = T // 512              # 16 token chunks
KT = E // 128                  # 8 contraction tiles
THETA = 10000.0
NEG_INF = -1e30

_prog_cache = {}


class _Unit:
    """One filler work unit: emit() puts ~0.5-2us of PE work (plus its DVE/
    DMA tail) into the instruction streams. due = (global tick) by which it
    must be emitted for correctness; budget pops usually emit it earlier."""

    __slots__ = ("emit", "due")

    def __init__(self, emit, due):
        self.emit = emit
        self.due = due


def _build_program(use_mask: bool):
    nc = bacc.Bacc()

    xT_d = nc.dram_tensor("xT", [E, T], BF16, kind="ExternalInput")
    cc_d = nc.dram_tensor("cc", [128, T], F32, kind="ExternalInput")
    ss_d = nc.dram_tensor("ss", [128, T], F32, kind="ExternalInput")
    wq_d = nc.dram_tensor("wqT", [E, CPC], BF16, kind="ExternalInput")
    wk_d = nc.dram_tensor("wkT", [E, CPC], BF16, kind="ExternalInput")
    wv_d = nc.dram_tensor("wvT", [E, CPC], BF16, kind="ExternalInput")
    wo_d = nc.dram_tensor("woT", [CPC, E], BF16, kind="ExternalInput")
    mb_d = None
    if use_mask:
        mb_d = nc.dram_tensor("mb", [128, B * 16], F32, kind="ExternalInput")
    y_d = nc.dram_tensor("yT", [E, T], BF16, kind="ExternalOutput")
    dbg_sc = dbg_pt = dbg_ocp = None
    if DEBUG_DUMP:
        dbg_sc = nc.dram_tensor("dbg_sc", [128, 16, 1024], F32,
                                kind="ExternalOutput")
        dbg_pt = nc.dram_tensor("dbg_pt", [128, 16, 1024], BF16,
                                kind="ExternalOutput")
        dbg_ocp = nc.dram_tensor("dbg_ocp", [130, 16, 512], F32,
                                 kind="ExternalOutput")

    xT_r = xT_d.rearrange("(k p) t -> p k t", p=128)
    wq_r = wq_d.rearrange("(k p) c -> p k c", p=128)
    wk_r = wk_d.rearrange("(k p) c -> p k c", p=128)
    wv_r = wv_d.rearrange("(k p) c -> p k c", p=128)
    wo_r = wo_d.rearrange("p (k c) -> p k c", c=128)

    with tile.TileContext(nc) as tc:
        with (
            tc.tile_pool(name="consts", bufs=1) as consts,
            tc.tile_pool(name="big", bufs=1) as big,
            tc.tile_pool(name="ph1", bufs=4) as ph1,
            tc.tile_pool(name="rope", bufs=2) as rope,
            tc.tile_pool(name="pt", bufs=4) as ptp,
            tc.tile_pool(name="norm", bufs=2) as norm,
            tc.tile_pool(name="ph3", bufs=3) as ph3,
            tc.tile_pool(name="psS", bufs=2, space="PSUM") as psS,
            tc.tile_pool(name="psA", bufs=2, space="PSUM") as psA,
            tc.tile_pool(name="psF", bufs=2, space="PSUM") as psF,
        ):
            # ---- constants ----
            wq_sb = consts.tile([128, KT, CPC], BF16)
            wk_sb = consts.tile([128, KT, CPC], BF16)
            wv_sb = consts.tile([128, KT, CPC], BF16)
            wo_sb = consts.tile([128, KT, 128], BF16)
            nc.sync.dma_start(wq_sb, wq_r)
            nc.scalar.dma_start(wk_sb, wk_r)
            nc.gpsimd.dma_start(wv_sb, wv_r)
            nc.scalar.dma_start(wo_sb, wo_r)
            mb_sb = None
            if use_mask:
                mb_sb = consts.tile([128, B * 16], F32)
                nc.sync.dma_start(mb_sb, mb_d[:, :])
            # bf16: an fp32 stationary would put the broadcast matmul in
            # fp32 LOW/HIGH mode (two ~750ns passes instead of one 213ns)
            ones65 = consts.tile([65, 64], BF16)
            nc.vector.memset(ones65, 1.0)

            # ---- persistent activations ----
            qT_sb = big.tile([128, NCHUNK, 512], BF16)
            kT_sb = big.tile([128, NCHUNK, 512], BF16)
            # vAB[:, ti, 0:65] = head-A v dims 0-63 + ones col 64;
            # vAB[:, ti, 65:130] = head-B v dims + ones col 129
            vAB_sb = big.tile([128, T // 128, 130], BF16)
            attnT_sb = big.tile([128, NCHUNK, 512], BF16)
            nc.vector.memset(vAB_sb[:, :, 64], 1.0)
            nc.vector.memset(vAB_sb[:, :, 129], 1.0)

            # ---------- phase-1 units (QKV projection of one 512-tok chunk) --
            def load_x(ch, prologue=False):
                # spread DMAs over independent queues: the Sync HWDGE queue
                # alone serializes ~1.3MB/chunk.  GpSimd rides the (idle)
                # SWDGE queue; the Scalar HWDGE queue joins only during the
                # prologue while the activation stream hasn't started.
                engs = ((nc.sync, nc.gpsimd, nc.scalar) if prologue
                        else (nc.sync, nc.gpsimd))
                xsb = ph1.tile([128, KT, 512], BF16, tag="xsb")
                for k in range(KT):   # one DMA per k-tile -> spread queues
                    engs[k % len(engs)].dma_start(
                        xsb[:, k, :], xT_r[:, k, ch * 512:(ch + 1) * 512])
                cc_sb = ph1.tile([128, 512], F32, tag="cc")
                ss_sb = ph1.tile([128, 512], F32, tag="ss")
                for i, h0 in enumerate((0, 256)):
                    engs[(i + 1) % len(engs)].dma_start(
                        cc_sb[:, h0:h0 + 256],
                        cc_d[:, ch * 512 + h0:ch * 512 + h0 + 256])
                    engs[i % len(engs)].dma_start(
                        ss_sb[:, h0:h0 + 256],
                        ss_d[:, ch * 512 + h0:ch * 512 + h0 + 256])
                return xsb, cc_sb, ss_sb

            def qk_proj(ch, tiles, w_sb, dstT):
                """8 accumulating matmuls + RoPE -> qT/kT chunk."""
                xsb, cc_sb, ss_sb = tiles
                ps = psF.tile([128, 512], F32, tag="f", name="ps_qk")
                for k in range(KT):
                    nc.tensor.matmul(ps, w_sb[:, k, :], xsb[:, k, :],
                                     start=(k == 0), stop=(k == KT - 1))
                t1 = rope.tile([128, 512], F32, tag="t1")
                t2 = rope.tile([128, 512], F32, tag="t2")
                t2sw = rope.tile([128, 512], F32, tag="t2sw")
                nc.vector.tensor_tensor(t1, ps, cc_sb, mybir.AluOpType.mult)
                nc.vector.tensor_tensor(t2, ps, ss_sb, mybir.AluOpType.mult)
                for b0 in (0, 64):
                    nc.sync.dma_start(t2sw[b0:b0 + 32], t2[b0 + 32:b0 + 64])
                    nc.sync.dma_start(t2sw[b0 + 32:b0 + 64], t2[b0:b0 + 32])
                nc.vector.tensor_tensor(dstT[:, ch, :], t1, t2sw,
                                        mybir.AluOpType.add)

            def v_proj(ch, tiles, half):
                """V for token blocks 2*half, 2*half+1 of chunk ch."""
                xsb, _, _ = tiles
                for tt in (2 * half, 2 * half + 1):
                    psv = psF.tile([128, 128], F32, tag="f", name="psv")
                    for k in range(KT):
                        nc.tensor.matmul(psv, xsb[:, k, tt * 128:(tt + 1) * 128],
                                         wv_sb[:, k, :],
                                         start=(k == 0), stop=(k == KT - 1))
                    ti = ch * 4 + tt
                    # one strided copy fills both heads' v slices
                    dst = vAB_sb[:, ti, :].rearrange(
                        "p (g c) -> p g c", g=2)[:, :, 0:64]
                    src = psv[:, :].rearrange("p (g c) -> p g c", g=2)
                    nc.vector.tensor_copy(dst, src)

            # ---------- phase-3 unit (output projection of one chunk) -------
            def out_proj(ch, cck4):
                """4 of the 8 wo k-tiles for chunk ch."""
                for cck in range(cck4 * 4, cck4 * 4 + 4):
                    psy = psF.tile([128, 512], F32, tag="f", name="psy")
                    nc.tensor.matmul(psy, wo_sb[:, cck, :], attnT_sb[:, ch, :],
                                     start=True, stop=True)
                    ysb = ph3.tile([128, 512], BF16, tag="ysb")
                    nc.vector.tensor_copy(ysb, psy)
                    # y writeback rides the idle GpSimd SWDGE queue so the
                    # Sync queue stays clear for latency-critical loads
                    nc.gpsimd.dma_start(
                        y_d[cck * 128:(cck + 1) * 128,
                            ch * 512:(ch + 1) * 512], ysb)

            # ---------- filler queue (kept sorted by due, FIFO on ties) ----
            queue = []
            _seq = [0]

            def enqueue(emit, due):
                import bisect
                _seq[0] += 1
                bisect.insort(queue, (due, _seq[0], _Unit(emit, due)))

            def pop_due(tick):
                while queue and queue[0][0] <= tick:
                    queue.pop(0)[2].emit()

            def pop_budget():
                if queue:
                    queue.pop(0)[2].emit()

            def enqueue_phase1(bn):
                """QKV units for all 4 chunks of batch bn.  kT / vA / vB / xsb
                feed matmul STATIONARY operands, and the PE's LDWEIGHTS
                pull-ahead reorder (64-deep window) does NOT respect the sems
                that gate the paired matmul — so all of load/k/v must be
                emitted >=2 blocks (>=64 PE instructions) before the first
                consuming block (bn, 0).  q feeds only MOVING operands (sem-
                gated properly), so q units may land as late as their block."""
                g0 = bn * 4 * 16        # tick of block (bn, 0), 16 ticks/block
                kv_due = g0 - 32
                tiles = {}
                # All 4 loads first, then all k, then all v, then all q:
                # v_proj consumes xsb as a matmul STATIONARY, and the PE's
                # LDWEIGHTS pull-ahead can read it up to ~64 instructions
                # early -- the k units (32 matmuls) in between guarantee the
                # DMA has landed before any v LDWEIGHTS can issue.
                for c4 in range(4):
                    ch = bn * 4 + c4

                    def em_load(ch=ch):
                        tiles[ch] = load_x(ch)
                    enqueue(em_load, kv_due)
                for c4 in range(4):
                    ch = bn * 4 + c4
                    enqueue(
                        lambda ch=ch: qk_proj(ch, tiles[ch], wk_sb, kT_sb),
                        kv_due + 1)
                for c4 in range(4):
                    ch = bn * 4 + c4
                    for half in (0, 1):
                        enqueue(
                            lambda ch=ch, half=half: v_proj(ch, tiles[ch], half),
                            kv_due + 2)
                # q last: frees this batch's xsb slots before the NEXT
                # batch's loads (sorted-queue order); q feeds only moving
                # operands, so closeness to its consumers is safe.
                for c4 in range(4):
                    ch = bn * 4 + c4
                    enqueue(
                        lambda ch=ch: qk_proj(ch, tiles[ch], wq_sb, qT_sb),
                        kv_due + 3)

            def enqueue_phase3(ch):
                for cck4 in (0, 1):
                    enqueue(lambda ch=ch, cck4=cck4: out_proj(ch, cck4), 1 << 30)

            # ---------- phase-2 block --------------------------------------
            def phase2_block(b, iq):
                g = (b * 4 + iq) * 16
                ch_i = b * 4 + iq
                posA = psA.tile([65, 512], F32, tag="pos", name="posA")
                posB = psA.tile([65, 512], F32, tag="pos", name="posB")
                pss = {}

                def scores(jb):
                    t = psS.tile([128, 1024], F32, tag="sc", name=f"sc{jb % 2}")
                    pss[jb] = t
                    ch_j = b * 4 + jb // 4
                    off_j = (jb % 4) * 128
                    for hh, b0 in ((0, 0), (1, 64)):
                        nc.tensor.matmul(
                            t[:, hh * 512:(hh + 1) * 512],
                            kT_sb[b0:b0 + 64, ch_j, off_j:off_j + 128],
                            qT_sb[b0:b0 + 64, ch_i, :],
                            start=True, stop=True, tile_position=(b0, 0))

                pop_due(g - 1)
                scores(0)
                scores(1)
                for jb in range(16):
                    pop_due(g + jb)
                    if jb == 1 and pending_fin:
                        # must emit before any pop_budget() can emit an
                        # out_proj that reads the attnT these fill in
                        for f in pending_fin:
                            f()
                        pending_fin.clear()
                    bias = (mb_sb[:, b * 16 + jb:b * 16 + jb + 1]
                            if use_mask else 0.0)
                    pT = ptp.tile([128, 1024], BF16, tag="pT", name="pT")
                    ptile = pss.pop(jb)
                    nc.scalar.activation(pT, ptile,
                                         mybir.ActivationFunctionType.Exp,
                                         bias=bias, scale=0.125)
                    if DEBUG_DUMP and b == 0 and iq == 0:
                        dsc = norm.tile([128, 1024], F32, tag="dbgsc")
                        nc.vector.tensor_copy(dsc, ptile)
                        nc.sync.dma_start(dbg_sc[:, jb, :], dsc)
                        nc.sync.dma_start(dbg_pt[:, jb, :], pT)
                    if jb + 2 < 16:
                        scores(jb + 2)
                    if jb % 2 == 1:
                        pop_budget()
                        if b == B - 1:
                            # no next-batch phase-1 exists; drain the
                            # out_proj backlog instead of leaving a tail
                            pop_budget()
                    for hh, pos in ((0, posA), (1, posB)):
                        nc.tensor.matmul(
                            pos,
                            vAB_sb[:, b * 16 + jb, hh * 65:(hh + 1) * 65],
                            pT[:, hh * 512:(hh + 1) * 512],
                            start=(jb == 0), stop=(jb == 15))
                # normalize: out = attn_out / colsum, written transposed
                # bf16.  ocp+csrow copies happen now (frees the pos banks);
                # the reciprocal, PE broadcast and multiply are deferred
                # into the next block so the PE never head-of-line waits on
                # the DVE.
                rows = []
                for hh, pos in ((0, posA), (1, posB)):
                    ocp = norm.tile([65, 512], F32, tag="ocp")
                    nc.vector.tensor_copy(ocp, pos)
                    # reciprocal_approx_fast needs a base-partition-0 input
                    # tile; feeding it a [64:65] slice mis-executes
                    csrow = norm.tile([1, 512], F32, tag="csrow")
                    nc.vector.tensor_copy(csrow, ocp[64:65, :])
                    rows.append((hh, ocp, csrow))

                def finish_norm():
                    for hh, ocp, csrow in rows:
                        csrec = norm.tile([1, 512], F32, tag="csrec")
                        nc.vector.reciprocal_approx_fast(out=csrec, in_=csrow)
                        # bf16 copy keeps the broadcast matmul out of fp32
                        # mode; the ~0.4% recip error scales whole columns
                        # uniformly and is negligible vs the 2e-2 budget
                        csrecb = norm.tile([1, 512], BF16, tag="csrecb")
                        nc.vector.tensor_copy(csrecb, csrec)
                        # broadcast recip to 64 partitions with a K=1 matmul
                        # (ones stationary x csrec moving)
                        csrep = psF.tile([64, 512], F32, tag="f",
                                         name="csrep")
                        nc.tensor.matmul(csrep, ones65[0:1, :], csrecb,
                                         start=True, stop=True)
                        if DEBUG_DUMP:
                            nc.sync.dma_start(
                                dbg_ocp[hh * 65:hh * 65 + 65, ch_i, :], ocp)
                        nc.vector.tensor_tensor(
                            attnT_sb[hh * 64:hh * 64 + 64, ch_i, :],
                            ocp[0:64, :], csrep, mybir.AluOpType.mult)
                if b == B - 1 and iq == 3:
                    finish_norm()   # last block: no next block to defer into
                else:
                    pending_fin.append(finish_norm)

            # ---------- program --------------------------------------------
            # prologue: all of batch 0's load/k/v inline (stationary-feeding
            # data must be far upstream of its consumers, see enqueue_phase1)
            t0 = {ch: load_x(ch, prologue=True) for ch in range(4)}
            for ch in range(4):
                qk_proj(ch, t0[ch], wk_sb, kT_sb)
            for ch in range(4):
                v_proj(ch, t0[ch], 0)
                v_proj(ch, t0[ch], 1)
            # only chunk 0's q is needed before block (0,0) starts (scores
            # consume qT as a sem-gated moving operand, so tight is safe);
            # q1-3 weave into the first blocks as filler
            qk_proj(0, t0[0], wq_sb, qT_sb)
            for ch in (1, 2, 3):
                enqueue(
                    lambda ch=ch: qk_proj(ch, t0[ch], wq_sb, qT_sb), ch - 1)

            pending_fin = []
            for b in range(B):
                if b + 1 < B:
                    enqueue_phase1(b + 1)
                for iq in range(4):
                    phase2_block(b, iq)
                    enqueue_phase3(b * 4 + iq)
            for f in pending_fin:
                f()
            pending_fin.clear()
            while queue:
                queue.pop(0)[2].emit()

    return nc


def _host_prep(x, key_padding_mask, dt, wq, wk, wv, wo):
    """Shared + per-core input arrays (all numpy)."""
    xT = np.ascontiguousarray(x.reshape(T, E).T).astype(ml_dtypes.bfloat16)

    # RoPE trig tables, rows [c;c;c;c] and [s;-s;s;-s] over 32-row blocks
    inv_freq = (1.0 / (THETA ** (np.arange(0, D, 2, dtype=np.float32) / D)))
    ang = dt.reshape(T).astype(np.float32)[None, :] * inv_freq[:, None]  # [32, T]
    cos = np.cos(ang).astype(np.float32)
    sin = np.sin(ang).astype(np.float32)
    cc = np.concatenate([cos, cos, cos, cos], axis=0)
    ssm = np.concatenate([sin, -sin, sin, -sin], axis=0)

    use_mask = bool(key_padding_mask.any())
    mb = None
    if use_mask:
        bias = np.where(key_padding_mask.reshape(T), NEG_INF, 0.0).astype(np.float32)
        # [128 j-in-block, B*16 block index]
        mb = np.ascontiguousarray(bias.reshape(B * 16, 128).T)

    # per-head channel permutation: [2r] then [2r+1] -> [r | 32+r]
    perm1 = np.concatenate([np.arange(0, D, 2), np.arange(1, D, 2)])

    per_core = []
    for c in range(NCORES):
        rows = []
        for h in range(c * HPC, (c + 1) * HPC):
            rows.append(h * D + perm1)
        rows = np.concatenate(rows)                      # permuted q/k rows
        rows_v = np.arange(c * CPC, (c + 1) * CPC)       # natural v rows
        # note: the 1/sqrt(D)=0.125 score scale is applied as the exp
        # activation's scale argument on device, not here
        wqT = np.ascontiguousarray(wq[rows].T).astype(ml_dtypes.bfloat16)
        wkT = np.ascontiguousarray(wk[rows].T).astype(ml_dtypes.bfloat16)
        wvT = np.ascontiguousarray(wv[rows_v].T).astype(ml_dtypes.bfloat16)
        woT = np.ascontiguousarray(wo[:, rows_v].T).astype(ml_dtypes.bfloat16)
        m = {"xT": xT, "cc": cc, "ss": ssm,
             "wqT": wqT, "wkT": wkT, "wvT": wvT, "woT": woT}
        if use_mask:
            m["mb"] = mb
        per_core.append(m)
    return per_core, use_mask


def kernel(x, key_padding_mask, dt, wq, wk, wv, wo, bo, _return_results=False):
    x = np.asarray(x, dtype=np.float32)
    key_padding_mask = np.asarray(key_padding_mask)
    dt = np.asarray(dt, dtype=np.float32)
    wq = np.asarray(wq, dtype=np.float32)
    wk = np.asarray(wk, dtype=np.float32)
    wv = np.asarray(wv, dtype=np.float32)
    wo = np.asarray(wo, dtype=np.float32)
    bo = np.asarray(bo, dtype=np.float32)

    in_maps, use_mask = _host_prep(x, key_padding_mask, dt, wq, wk, wv, wo)

    key = use_mask
    if key not in _prog_cache:
        prog = _build_program(use_mask)
        prog.finalize()
        _prog_cache[key] = prog
    nc = _prog_cache[key]

    res = run_bass_kernel_spmd(nc, in_maps, list(range(NCORES)))

    y = np.zeros((E, T), dtype=np.float32)
    for r in res.results:
        y += r["yT"].astype(np.float32)
    out = (y.T + bo[None, :]).reshape(B, S, E).astype(np.float32)
    if _return_results:
        return out, res
    return out



# revision 24
# speedup vs baseline: 1.2978x; 1.0032x over previous
"""MHA + RoPE fused kernel for Trainium2, sharded tensor-parallel over heads
across 8 NeuronCores.

Problem (hardcoded): B=4, S=2048, E=1024, H=16 heads, D=64.
  xq = x @ wq.T ; xk = x @ wk.T ; xv = x @ wv.T          [B,S,H,D]
  RoPE(xq, xk) with angles dt[b,s] * inv_freq[r]
  scores = softmax(xq @ xk.T / sqrt(D) + mask)            per (b, head)
  out = (scores @ xv) reshaped to [B,S,E]; y = out @ wo.T + bo
Each core owns 2 heads (128 q/k/v channels) and the matching 128 rows of
wo.T; it computes a full partial y (row-parallel output projection) and the
host sums the 8 bf16 partials.

Schedule: attention phase-2 runs in 16 blocks of (batch b, 512-token query
chunk iq).  Inside a block the 16 key-tile loop is software-pipelined: the
PE computes scores two key-tiles ahead of the ScalarE exp, and attn@V
trails right behind its exp, so the ScalarE (~286us of exp work) never
starves and the PE never head-of-line blocks.  QKV projections of the next
batch and output projections of finished chunks are chopped into ~1-2us
"filler" units and woven between key-tiles with deadline tags, keeping the
PE continuously busy so the HAM clock gate stays at K=8/8.

vs. the 524us baseline (now ~460us):
 - softmax normalize is split: the pos->SBUF copies run at block end, but
   reciprocal + K=1-broadcast-matmul + multiply are deferred into the next
   block so the PE never waits on the DVE;
 - the broadcast matmul runs in bf16 (an fp32 ones vector put it in fp32
   LOW/HIGH mode: two ~750ns passes instead of one ~230ns - 45us of PE);
 - vA/vB live interleaved in one [128, ti, 130] tile so each v-projection
   drains with one strided copy instead of two;
 - only chunk 0's q-projection precedes block (0,0); q1-3 weave into the
   first block as filler (shorter prologue);
 - DMAs are spread across the Sync HWDGE, Scalar HWDGE (prologue only) and
   GpSimd SWDGE queues; y-writeback rides GpSimd so loads never queue
   behind it;
 - the last batch drains the out_proj backlog with doubled budget pops.

fp8 (DoubleRow) was evaluated and rejected: quantizing any single tensor
of the attention path (pT, V, attnT, or wo) to e4m3 already exceeds the
2e-2 max-rel-error budget (pT alone: 0.16).

PSUM budget (8 banks): 2x scores [128,1024]f32 (4) + 2x attn-out [65,512]
accumulators (2) + 2x filler tiles [128,512] (2).
"""

import os
import sys

sys.path.insert(0, "/opt/trn_rl_repo")

import numpy as np
import ml_dtypes

DEBUG_DUMP = bool(os.environ.get("KERNEL_DEBUG_DUMP"))

import concourse.bass as bass
from concourse import bacc
import concourse.tile as tile
from concourse import mybir
from concourse.bass_utils import run_bass_kernel_spmd

F32 = mybir.dt.float32
BF16 = mybir.dt.bfloat16

B, S, E, H, D = 4, 2048, 1024, 16, 64
T = B * S                      # 8192 flattened tokens
NCORES = 8
HPC = H // NCORES              # 2 heads per core
CPC = HPC * D                  # 128 channels per core
NCHUNK = T // 512              # 16 token chunks
KT = E // 128                  # 8 contraction tiles
THETA = 10000.0
NEG_INF = -1e30

_prog_cache = {}


class _Unit:
    """One filler work unit: emit() puts ~0.5-2us of PE work (plus its DVE/
    DMA tail) into the instruction streams. due = (global tick) by which it
    must be emitted for correctness; budget pops usually emit it earlier."""

    __slots__ = ("emit", "due")

    def __init__(self, emit, due):
        self.emit = emit
        self.due = due


def _build_program(use_mask: bool):
    nc = bacc.Bacc()

    xT_d = nc.dram_tensor("xT", [E, T], BF16, kind="ExternalInput")
    cc_d = nc.dram_tensor("cc", [128, T], F32, kind="ExternalInput")
    ss_d = nc.dram_tensor("ss", [128, T], F32, kind="ExternalInput")
    wq_d = nc.dram_tensor("wqT", [E, CPC], BF16, kind="ExternalInput")
    wk_d = nc.dram_tensor("wkT", [E, CPC], BF16, kind="ExternalInput")
    wv_d = nc.dram_tensor("wvT", [E, CPC], BF16, kind="ExternalInput")
    wo_d = nc.dram_tensor("woT", [CPC, E], BF16, kind="ExternalInput")
    mb_d = None
    if use_mask:
        mb_d = nc.dram_tensor("mb", [128, B * 16], F32, kind="ExternalInput")
    y_d = nc.dram_tensor("yT", [E, T], BF16, kind="ExternalOutput")
    dbg_sc = dbg_pt = dbg_ocp = None
    if DEBUG_DUMP:
        dbg_sc = nc.dram_tensor("dbg_sc", [128, 16, 1024], F32,
                                kind="ExternalOutput")
        dbg_pt = nc.dram_tensor("dbg_pt", [128, 16, 1024], BF16,
                                kind="ExternalOutput")
        dbg_ocp = nc.dram_tensor("dbg_ocp", [130, 16, 512], F32,
                                 kind="ExternalOutput")

    xT_r = xT_d.rearrange("(k p) t -> p k t", p=128)
    wq_r = wq_d.rearrange("(k p) c -> p k c", p=128)
    wk_r = wk_d.rearrange("(k p) c -> p k c", p=128)
    wv_r = wv_d.rearrange("(k p) c -> p k c", p=128)
    wo_r = wo_d.rearrange("p (k c) -> p k c", c=128)

    with tile.TileContext(nc) as tc:
        with (
            tc.tile_pool(name="consts", bufs=1) as consts,
            tc.tile_pool(name="big", bufs=1) as big,
            tc.tile_pool(name="ph1", bufs=4) as ph1,
            tc.tile_pool(name="rope", bufs=2) as rope,
            tc.tile_pool(name="pt", bufs=4) as ptp,
            tc.tile_pool(name="norm", bufs=2) as norm,
            tc.tile_pool(name="ph3", bufs=3) as ph3,
            tc.tile_pool(name="psS", bufs=2, space="PSUM") as psS,
            tc.tile_pool(name="psA", bufs=2, space="PSUM") as psA,
            tc.tile_pool(name="psF", bufs=2, space="PSUM") as psF,
        ):
            # ---- constants ----
            wq_sb = consts.tile([128, KT, CPC], BF16)
            wk_sb = consts.tile([128, KT, CPC], BF16)
            wv_sb = consts.tile([128, KT, CPC], BF16)
            wo_sb = consts.tile([128, KT, 128], BF16)
            nc.sync.dma_start(wq_sb, wq_r)
            nc.scalar.dma_start(wk_sb, wk_r)
            nc.gpsimd.dma_start(wv_sb, wv_r)
            nc.scalar.dma_start(wo_sb, wo_r)
            mb_sb = None
            if use_mask:
                mb_sb = consts.tile([128, B * 16], F32)
                nc.sync.dma_start(mb_sb, mb_d[:, :])
            # bf16: an fp32 stationary would put the broadcast matmul in
            # fp32 LOW/HIGH mode (two ~750ns passes instead of one 213ns)
            ones65 = consts.tile([65, 64], BF16)
            nc.vector.memset(ones65, 1.0)

            # ---- persistent activations ----
            qT_sb = big.tile([128, NCHUNK, 512], BF16)
            kT_sb = big.tile([128, NCHUNK, 512], BF16)
            # vAB[:, ti, 0:65] = head-A v dims 0-63 + ones col 64;
            # vAB[:, ti, 65:130] = head-B v dims + ones col 129
            vAB_sb = big.tile([128, T // 128, 130], BF16)
            attnT_sb = big.tile([128, NCHUNK, 512], BF16)
            nc.vector.memset(vAB_sb[:, :, 64], 1.0)
            nc.vector.memset(vAB_sb[:, :, 129], 1.0)

            # ---------- phase-1 units (QKV projection of one 512-tok chunk) --
            def load_x(ch, prologue=False):
                # spread DMAs over independent queues: the Sync HWDGE queue
                # alone serializes ~1.3MB/chunk.  GpSimd rides the (idle)
                # SWDGE queue; the Scalar HWDGE queue joins only during the
                # prologue while the activation stream hasn't started.
                engs = ((nc.sync, nc.gpsimd, nc.scalar) if prologue
                        else (nc.sync, nc.gpsimd))
                xsb = ph1.tile([128, KT, 512], BF16, tag="xsb")
                for k in range(KT):   # one DMA per k-tile -> spread queues
                    engs[k % len(engs)].dma_start(
                        xsb[:, k, :], xT_r[:, k, ch * 512:(ch + 1) * 512])
                cc_sb = ph1.tile([128, 512], F32, tag="cc")
                ss_sb = ph1.tile([128, 512], F32, tag="ss")
                for i, h0 in enumerate((0, 256)):
                    engs[(i + 1) % len(engs)].dma_start(
                        cc_sb[:, h0:h0 + 256],
                        cc_d[:, ch * 512 + h0:ch * 512 + h0 + 256])
                    engs[i % len(engs)].dma_start(
                        ss_sb[:, h0:h0 + 256],
                        ss_d[:, ch * 512 + h0:ch * 512 + h0 + 256])
                return xsb, cc_sb, ss_sb

            def qk_proj(ch, tiles, w_sb, dstT):
                """8 accumulating matmuls + RoPE -> qT/kT chunk."""
                xsb, cc_sb, ss_sb = tiles
                ps = psF.tile([128, 512], F32, tag="f", name="ps_qk")
                for k in range(KT):
                    nc.tensor.matmul(ps, w_sb[:, k, :], xsb[:, k, :],
                                     start=(k == 0), stop=(k == KT - 1))
                t1 = rope.tile([128, 512], F32, tag="t1")
                t2 = rope.tile([128, 512], F32, tag="t2")
                t2sw = rope.tile([128, 512], F32, tag="t2sw")
                nc.vector.tensor_tensor(t1, ps, cc_sb, mybir.AluOpType.mult)
                nc.vector.tensor_tensor(t2, ps, ss_sb, mybir.AluOpType.mult)
                for b0 in (0, 64):
                    nc.sync.dma_start(t2sw[b0:b0 + 32], t2[b0 + 32:b0 + 64])
                    nc.sync.dma_start(t2sw[b0 + 32:b0 + 64], t2[b0:b0 + 32])
                nc.vector.tensor_tensor(dstT[:, ch, :], t1, t2sw,
                                        mybir.AluOpType.add)

            def v_proj(ch, tiles, half):
                """V for token blocks 2*half, 2*half+1 of chunk ch."""
                xsb, _, _ = tiles
                for tt in (2 * half, 2 * half + 1):
                    psv = psF.tile([128, 128], F32, tag="f", name="psv")
                    for k in range(KT):
                        nc.tensor.matmul(psv, xsb[:, k, tt * 128:(tt + 1) * 128],
                                         wv_sb[:, k, :],
                                         start=(k == 0), stop=(k == KT - 1))
                    ti = ch * 4 + tt
                    # one strided copy fills both heads' v slices
                    dst = vAB_sb[:, ti, :].rearrange(
                        "p (g c) -> p g c", g=2)[:, :, 0:64]
                    src = psv[:, :].rearrange("p (g c) -> p g c", g=2)
                    nc.vector.tensor_copy(dst, src)

            # ---------- phase-3 unit (output projection of one chunk) -------
            def out_proj(ch, cck4):
                """4 of the 8 wo k-tiles for chunk ch."""
                for cck in range(cck4 * 4, cck4 * 4 + 4):
                    psy = psF.tile([128, 512], F32, tag="f", name="psy")
                    nc.tensor.matmul(psy, wo_sb[:, cck, :], attnT_sb[:, ch, :],
                                     start=True, stop=True)
                    ysb = ph3.tile([128, 512], BF16, tag="ysb")
                    nc.vector.tensor_copy(ysb, psy)
                    # y writeback rides the idle GpSimd SWDGE queue so the
                    # Sync queue stays clear for latency-critical loads; the
                    # last chunks go back to Sync (idle by then, and SWDGE's
                    # ~1us/DMA issue overhead would stretch the tail)
                    eng = nc.sync if ch >= NCHUNK - 2 else nc.gpsimd
                    eng.dma_start(
                        y_d[cck * 128:(cck + 1) * 128,
                            ch * 512:(ch + 1) * 512], ysb)

            # ---------- filler queue (kept sorted by due, FIFO on ties) ----
            queue = []
            _seq = [0]

            def enqueue(emit, due):
                import bisect
                _seq[0] += 1
                bisect.insort(queue, (due, _seq[0], _Unit(emit, due)))

            def pop_due(tick):
                while queue and queue[0][0] <= tick:
                    queue.pop(0)[2].emit()

            def pop_budget():
                if queue:
                    queue.pop(0)[2].emit()

            def enqueue_phase1(bn):
                """QKV units for all 4 chunks of batch bn.  kT / vA / vB / xsb
                feed matmul STATIONARY operands, and the PE's LDWEIGHTS
                pull-ahead reorder (64-deep window) does NOT respect the sems
                that gate the paired matmul — so all of load/k/v must be
                emitted >=2 blocks (>=64 PE instructions) before the first
                consuming block (bn, 0).  q feeds only MOVING operands (sem-
                gated properly), so q units may land as late as their block."""
                g0 = bn * 4 * 16        # tick of block (bn, 0), 16 ticks/block
                kv_due = g0 - 32
                tiles = {}
                # All 4 loads first, then all k, then all v, then all q:
                # v_proj consumes xsb as a matmul STATIONARY, and the PE's
                # LDWEIGHTS pull-ahead can read it up to ~64 instructions
                # early -- the k units (32 matmuls) in between guarantee the
                # DMA has landed before any v LDWEIGHTS can issue.
                for c4 in range(4):
                    ch = bn * 4 + c4

                    def em_load(ch=ch):
                        tiles[ch] = load_x(ch)
                    enqueue(em_load, kv_due)
                for c4 in range(4):
                    ch = bn * 4 + c4
                    enqueue(
                        lambda ch=ch: qk_proj(ch, tiles[ch], wk_sb, kT_sb),
                        kv_due + 1)
                for c4 in range(4):
                    ch = bn * 4 + c4
                    for half in (0, 1):
                        enqueue(
                            lambda ch=ch, half=half: v_proj(ch, tiles[ch], half),
                            kv_due + 2)
                # q last: frees this batch's xsb slots before the NEXT
                # batch's loads (sorted-queue order); q feeds only moving
                # operands, so closeness to its consumers is safe.
                for c4 in range(4):
                    ch = bn * 4 + c4
                    enqueue(
                        lambda ch=ch: qk_proj(ch, tiles[ch], wq_sb, qT_sb),
                        kv_due + 3)

            def enqueue_phase3(ch):
                for cck4 in (0, 1):
                    enqueue(lambda ch=ch, cck4=cck4: out_proj(ch, cck4), 1 << 30)

            # ---------- phase-2 block --------------------------------------
            def phase2_block(b, iq):
                g = (b * 4 + iq) * 16
                ch_i = b * 4 + iq
                posA = psA.tile([65, 512], F32, tag="pos", name="posA")
                posB = psA.tile([65, 512], F32, tag="pos", name="posB")
                pss = {}

                def scores(jb):
                    t = psS.tile([128, 1024], F32, tag="sc", name=f"sc{jb % 2}")
                    pss[jb] = t
                    ch_j = b * 4 + jb // 4
                    off_j = (jb % 4) * 128
                    for hh, b0 in ((0, 0), (1, 64)):
                        nc.tensor.matmul(
                            t[:, hh * 512:(hh + 1) * 512],
                            kT_sb[b0:b0 + 64, ch_j, off_j:off_j + 128],
                            qT_sb[b0:b0 + 64, ch_i, :],
                            start=True, stop=True, tile_position=(b0, 0))

                pop_due(g - 1)
                scores(0)
                scores(1)
                for jb in range(16):
                    pop_due(g + jb)
                    if jb == 1 and pending_fin:
                        # must emit before any pop_budget() can emit an
                        # out_proj that reads the attnT these fill in
                        for f in pending_fin:
                            f()
                        pending_fin.clear()
                    bias = (mb_sb[:, b * 16 + jb:b * 16 + jb + 1]
                            if use_mask else 0.0)
                    pT = ptp.tile([128, 1024], BF16, tag="pT", name="pT")
                    ptile = pss.pop(jb)
                    nc.scalar.activation(pT, ptile,
                                         mybir.ActivationFunctionType.Exp,
                                         bias=bias, scale=0.125)
                    if DEBUG_DUMP and b == 0 and iq == 0:
                        dsc = norm.tile([128, 1024], F32, tag="dbgsc")
                        nc.vector.tensor_copy(dsc, ptile)
                        nc.sync.dma_start(dbg_sc[:, jb, :], dsc)
                        nc.sync.dma_start(dbg_pt[:, jb, :], pT)
                    if jb + 2 < 16:
                        scores(jb + 2)
                    if jb % 2 == 1:
                        pop_budget()
                        if b == B - 1:
                            # no next-batch phase-1 exists; drain the
                            # out_proj backlog instead of leaving a tail
                            pop_budget()
                    for hh, pos in ((0, posA), (1, posB)):
                        nc.tensor.matmul(
                            pos,
                            vAB_sb[:, b * 16 + jb, hh * 65:(hh + 1) * 65],
                            pT[:, hh * 512:(hh + 1) * 512],
                            start=(jb == 0), stop=(jb == 15))
                # normalize: out = attn_out / colsum, written transposed
                # bf16.  ocp+csrow copies happen now (frees the pos banks);
                # the reciprocal, PE broadcast and multiply are deferred
                # into the next block so the PE never head-of-line waits on
                # the DVE.
                rows = []
                for hh, pos in ((0, posA), (1, posB)):
                    ocp = norm.tile([65, 512], F32, tag="ocp")
                    nc.vector.tensor_copy(ocp, pos)
                    # reciprocal_approx_fast needs a base-partition-0 input
                    # tile; feeding it a [64:65] slice mis-executes
                    csrow = norm.tile([1, 512], F32, tag="csrow")
                    nc.vector.tensor_copy(csrow, ocp[64:65, :])
                    rows.append((hh, ocp, csrow))

                def finish_norm():
                    for hh, ocp, csrow in rows:
                        csrec = norm.tile([1, 512], F32, tag="csrec")
                        nc.vector.reciprocal_approx_fast(out=csrec, in_=csrow)
                        # bf16 copy keeps the broadcast matmul out of fp32
                        # mode; the ~0.4% recip error scales whole columns
                        # uniformly and is negligible vs the 2e-2 budget
                        csrecb = norm.tile([1, 512], BF16, tag="csrecb")
                        nc.vector.tensor_copy(csrecb, csrec)
                        # broadcast recip to 64 partitions with a K=1 matmul
                        # (ones stationary x csrec moving)
                        csrep = psF.tile([64, 512], F32, tag="f",
                                         name="csrep")
                        nc.tensor.matmul(csrep, ones65[0:1, :], csrecb,
                                         start=True, stop=True)
                        if DEBUG_DUMP:
                            nc.sync.dma_start(
                                dbg_ocp[hh * 65:hh * 65 + 65, ch_i, :], ocp)
                        nc.vector.tensor_tensor(
                            attnT_sb[hh * 64:hh * 64 + 64, ch_i, :],
                            ocp[0:64, :], csrep, mybir.AluOpType.mult)
                if b == B - 1 and iq == 3:
                    finish_norm()   # last block: no next block to defer into
                else:
                    pending_fin.append(finish_norm)

            # ---------- program --------------------------------------------
            # prologue: all of batch 0's load/k/v inline (stationary-feeding
            # data must be far upstream of its consumers, see enqueue_phase1)
            t0 = {ch: load_x(ch, prologue=True) for ch in range(4)}
            for ch in range(4):
                qk_proj(ch, t0[ch], wk_sb, kT_sb)
            for ch in range(4):
                v_proj(ch, t0[ch], 0)
                v_proj(ch, t0[ch], 1)
            # only chunk 0's q is needed before block (0,0) starts (scores
            # consume qT as a sem-gated moving operand, so tight is safe);
            # q1-3 weave into the first blocks as filler
            qk_proj(0, t0[0], wq_sb, qT_sb)
            for ch in (1, 2, 3):
                enqueue(
                    lambda ch=ch: qk_proj(ch, t0[ch], wq_sb, qT_sb), ch - 1)

            pending_fin = []
            for b in range(B):
                if b + 1 < B:
                    enqueue_phase1(b + 1)
                for iq in range(4):
                    phase2_block(b, iq)
                    enqueue_phase3(b * 4 + iq)
            for f in pending_fin:
                f()
            pending_fin.clear()
            while queue:
                queue.pop(0)[2].emit()

    return nc


def _host_prep(x, key_padding_mask, dt, wq, wk, wv, wo):
    """Shared + per-core input arrays (all numpy)."""
    xT = np.ascontiguousarray(x.reshape(T, E).T).astype(ml_dtypes.bfloat16)

    # RoPE trig tables, rows [c;c;c;c] and [s;-s;s;-s] over 32-row blocks
    inv_freq = (1.0 / (THETA ** (np.arange(0, D, 2, dtype=np.float32) / D)))
    ang = dt.reshape(T).astype(np.float32)[None, :] * inv_freq[:, None]  # [32, T]
    cos = np.cos(ang).astype(np.float32)
    sin = np.sin(ang).astype(np.float32)
    cc = np.concatenate([cos, cos, cos, cos], axis=0)
    ssm = np.concatenate([sin, -sin, sin, -sin], axis=0)

    use_mask = bool(key_padding_mask.any())
    mb = None
    if use_mask:
        bias = np.where(key_padding_mask.reshape(T), NEG_INF, 0.0).astype(np.float32)
        # [128 j-in-block, B*16 block index]
        mb = np.ascontiguousarray(bias.reshape(B * 16, 128).T)

    # per-head channel permutation: [2r] then [2r+1] -> [r | 32+r]
    perm1 = np.concatenate([np.arange(0, D, 2), np.arange(1, D, 2)])

    per_core = []
    for c in range(NCORES):
        rows = []
        for h in range(c * HPC, (c + 1) * HPC):
            rows.append(h * D + perm1)
        rows = np.concatenate(rows)                      # permuted q/k rows
        rows_v = np.arange(c * CPC, (c + 1) * CPC)       # natural v rows
        # note: the 1/sqrt(D)=0.125 score scale is applied as the exp
        # activation's scale argument on device, not here
        wqT = np.ascontiguousarray(wq[rows].T).astype(ml_dtypes.bfloat16)
        wkT = np.ascontiguousarray(wk[rows].T).astype(ml_dtypes.bfloat16)
        wvT = np.ascontiguousarray(wv[rows_v].T).astype(ml_dtypes.bfloat16)
        woT = np.ascontiguousarray(wo[:, rows_v].T).astype(ml_dtypes.bfloat16)
        m = {"xT": xT, "cc": cc, "ss": ssm,
             "wqT": wqT, "wkT": wkT, "wvT": wvT, "woT": woT}
        if use_mask:
            m["mb"] = mb
        per_core.append(m)
    return per_core, use_mask


def kernel(x, key_padding_mask, dt, wq, wk, wv, wo, bo, _return_results=False):
    x = np.asarray(x, dtype=np.float32)
    key_padding_mask = np.asarray(key_padding_mask)
    dt = np.asarray(dt, dtype=np.float32)
    wq = np.asarray(wq, dtype=np.float32)
    wk = np.asarray(wk, dtype=np.float32)
    wv = np.asarray(wv, dtype=np.float32)
    wo = np.asarray(wo, dtype=np.float32)
    bo = np.asarray(bo, dtype=np.float32)

    in_maps, use_mask = _host_prep(x, key_padding_mask, dt, wq, wk, wv, wo)

    key = use_mask
    if key not in _prog_cache:
        prog = _build_program(use_mask)
        prog.finalize()
        _prog_cache[key] = prog
    nc = _prog_cache[key]

    res = run_bass_kernel_spmd(nc, in_maps, list(range(NCORES)))

    y = np.zeros((E, T), dtype=np.float32)
    for r in res.results:
        y += r["yT"].astype(np.float32)
    out = (y.T + bo[None, :]).reshape(B, S, E).astype(np.float32)
    if _return_results:
        return out, res
    return out



# revision 25
# speedup vs baseline: 1.3107x; 1.0099x over previous
"""MHA + RoPE fused kernel for Trainium2, sharded tensor-parallel over heads
across 8 NeuronCores.

Problem (hardcoded): B=4, S=2048, E=1024, H=16 heads, D=64.
  xq = x @ wq.T ; xk = x @ wk.T ; xv = x @ wv.T          [B,S,H,D]
  RoPE(xq, xk) with angles dt[b,s] * inv_freq[r]
  scores = softmax(xq @ xk.T / sqrt(D) + mask)            per (b, head)
  out = (scores @ xv) reshaped to [B,S,E]; y = out @ wo.T + bo
Each core owns 2 heads (128 q/k/v channels) and the matching 128 rows of
wo.T; it computes a full partial y (row-parallel output projection) and the
host sums the 8 bf16 partials.

Schedule: attention phase-2 runs in 16 blocks of (batch b, 512-token query
chunk iq).  Inside a block the 16 key-tile loop is software-pipelined: the
PE computes scores two key-tiles ahead of the ScalarE exp, and attn@V
trails right behind its exp, so the ScalarE (~286us of exp work) never
starves and the PE never head-of-line blocks.  QKV projections of the next
batch and output projections of finished chunks are chopped into ~1-2us
"filler" units and woven between key-tiles with deadline tags, keeping the
PE continuously busy so the HAM clock gate stays at K=8/8.

vs. the 524us baseline (now ~460us):
 - softmax normalize is split: the pos->SBUF copies run at block end, but
   reciprocal + K=1-broadcast-matmul + multiply are deferred into the next
   block so the PE never waits on the DVE;
 - the broadcast matmul runs in bf16 (an fp32 ones vector put it in fp32
   LOW/HIGH mode: two ~750ns passes instead of one ~230ns - 45us of PE);
 - vA/vB live interleaved in one [128, ti, 130] tile so each v-projection
   drains with one strided copy instead of two;
 - only chunk 0's q-projection precedes block (0,0); q1-3 weave into the
   first block as filler (shorter prologue);
 - DMAs are spread across the Sync HWDGE, Scalar HWDGE (prologue only) and
   GpSimd SWDGE queues; y-writeback rides GpSimd so loads never queue
   behind it;
 - the last batch drains the out_proj backlog with doubled budget pops.

fp8 (DoubleRow) was evaluated and rejected: quantizing any single tensor
of the attention path (pT, V, attnT, or wo) to e4m3 already exceeds the
2e-2 max-rel-error budget (pT alone: 0.16).

PSUM budget (8 banks): 2x scores [128,1024]f32 (4) + 2x attn-out [65,512]
accumulators (2) + 2x filler tiles [128,512] (2).
"""

import os
import sys

sys.path.insert(0, "/opt/trn_rl_repo")

import numpy as np
import ml_dtypes

DEBUG_DUMP = bool(os.environ.get("KERNEL_DEBUG_DUMP"))

import concourse.bass as bass
from concourse import bacc
import concourse.tile as tile
from concourse import mybir
from concourse.bass_utils import run_bass_kernel_spmd

F32 = mybir.dt.float32
BF16 = mybir.dt.bfloat16

B, S, E, H, D = 4, 2048, 1024, 16, 64
T = B * S                      # 8192 flattened tokens
NCORES = 8
HPC = H // NCORES              # 2 heads per core
CPC = HPC * D                  # 128 channels per core
NCHUNK = T // 512              # 16 token chunks
KT = E // 128                  # 8 contraction tiles
THETA = 10000.0
NEG_INF = -1e30

_prog_cache = {}


class _Unit:
    """One filler work unit: emit() puts ~0.5-2us of PE work (plus its DVE/
    DMA tail) into the instruction streams. due = (global tick) by which it
    must be emitted for correctness; budget pops usually emit it earlier."""

    __slots__ = ("emit", "due")

    def __init__(self, emit, due):
        self.emit = emit
        self.due = due


def _build_program(use_mask: bool):
    nc = bacc.Bacc()

    xT_d = nc.dram_tensor("xT", [E, T], BF16, kind="ExternalInput")
    cc_d = nc.dram_tensor("cc", [128, T], F32, kind="ExternalInput")
    ss_d = nc.dram_tensor("ss", [128, T], F32, kind="ExternalInput")
    wq_d = nc.dram_tensor("wqT", [E, CPC], BF16, kind="ExternalInput")
    wk_d = nc.dram_tensor("wkT", [E, CPC], BF16, kind="ExternalInput")
    wv_d = nc.dram_tensor("wvT", [E, CPC], BF16, kind="ExternalInput")
    wo_d = nc.dram_tensor("woT", [CPC, E], BF16, kind="ExternalInput")
    mb_d = None
    if use_mask:
        mb_d = nc.dram_tensor("mb", [128, B * 16], F32, kind="ExternalInput")
    y_d = nc.dram_tensor("yT", [E, T], BF16, kind="ExternalOutput")
    dbg_sc = dbg_pt = dbg_ocp = None
    if DEBUG_DUMP:
        dbg_sc = nc.dram_tensor("dbg_sc", [128, 16, 1024], F32,
                                kind="ExternalOutput")
        dbg_pt = nc.dram_tensor("dbg_pt", [128, 16, 1024], BF16,
                                kind="ExternalOutput")
        dbg_ocp = nc.dram_tensor("dbg_ocp", [130, 16, 512], F32,
                                 kind="ExternalOutput")

    xT_r = xT_d.rearrange("(k p) t -> p k t", p=128)
    wq_r = wq_d.rearrange("(k p) c -> p k c", p=128)
    wk_r = wk_d.rearrange("(k p) c -> p k c", p=128)
    wv_r = wv_d.rearrange("(k p) c -> p k c", p=128)
    wo_r = wo_d.rearrange("p (k c) -> p k c", c=128)

    with tile.TileContext(nc) as tc:
        with (
            tc.tile_pool(name="consts", bufs=1) as consts,
            tc.tile_pool(name="big", bufs=1) as big,
            tc.tile_pool(name="ph1", bufs=4) as ph1,
            tc.tile_pool(name="rope", bufs=2) as rope,
            tc.tile_pool(name="pt", bufs=4) as ptp,
            tc.tile_pool(name="norm", bufs=2) as norm,
            tc.tile_pool(name="ph3", bufs=3) as ph3,
            tc.tile_pool(name="psS", bufs=2, space="PSUM") as psS,
            tc.tile_pool(name="psA", bufs=2, space="PSUM") as psA,
            tc.tile_pool(name="psF", bufs=2, space="PSUM") as psF,
        ):
            # ---- constants ----
            wq_sb = consts.tile([128, KT, CPC], BF16)
            wk_sb = consts.tile([128, KT, CPC], BF16)
            wv_sb = consts.tile([128, KT, CPC], BF16)
            wo_sb = consts.tile([128, KT, 128], BF16)
            nc.sync.dma_start(wq_sb, wq_r)
            nc.scalar.dma_start(wk_sb, wk_r)
            nc.gpsimd.dma_start(wv_sb, wv_r)
            nc.scalar.dma_start(wo_sb, wo_r)
            mb_sb = None
            if use_mask:
                mb_sb = consts.tile([128, B * 16], F32)
                nc.sync.dma_start(mb_sb, mb_d[:, :])
            # bf16: an fp32 stationary would put the broadcast matmul in
            # fp32 LOW/HIGH mode (two ~750ns passes instead of one 213ns)
            ones65 = consts.tile([65, 64], BF16)
            nc.vector.memset(ones65, 1.0)

            # ---- persistent activations ----
            qT_sb = big.tile([128, NCHUNK, 512], BF16)
            kT_sb = big.tile([128, NCHUNK, 512], BF16)
            # vAB[:, ti, 0:65] = head-A v dims 0-63 + ones col 64;
            # vAB[:, ti, 65:130] = head-B v dims + ones col 129
            vAB_sb = big.tile([128, T // 128, 130], BF16)
            attnT_sb = big.tile([128, NCHUNK, 512], BF16)
            nc.vector.memset(vAB_sb[:, :, 64], 1.0)
            nc.vector.memset(vAB_sb[:, :, 129], 1.0)

            # ---------- phase-1 units (QKV projection of one 512-tok chunk) --
            def load_x(ch, prologue=False):
                # spread DMAs over independent queues: the Sync HWDGE queue
                # alone serializes ~1.3MB/chunk.  GpSimd rides the (idle)
                # SWDGE queue; the Scalar HWDGE queue joins only during the
                # prologue while the activation stream hasn't started.
                engs = ((nc.sync, nc.gpsimd, nc.scalar) if prologue
                        else (nc.sync, nc.gpsimd))
                xsb = ph1.tile([128, KT, 512], BF16, tag="xsb")
                for k in range(KT):   # one DMA per k-tile -> spread queues
                    engs[k % len(engs)].dma_start(
                        xsb[:, k, :], xT_r[:, k, ch * 512:(ch + 1) * 512])
                cc_sb = ph1.tile([128, 512], F32, tag="cc")
                ss_sb = ph1.tile([128, 512], F32, tag="ss")
                for i, h0 in enumerate((0, 256)):
                    engs[(i + 1) % len(engs)].dma_start(
                        cc_sb[:, h0:h0 + 256],
                        cc_d[:, ch * 512 + h0:ch * 512 + h0 + 256])
                    engs[i % len(engs)].dma_start(
                        ss_sb[:, h0:h0 + 256],
                        ss_d[:, ch * 512 + h0:ch * 512 + h0 + 256])
                return xsb, cc_sb, ss_sb

            def qk_proj(ch, tiles, w_sb, dstT):
                """8 accumulating matmuls + RoPE -> qT/kT chunk."""
                xsb, cc_sb, ss_sb = tiles
                ps = psF.tile([128, 512], F32, tag="f", name="ps_qk")
                for k in range(KT):
                    nc.tensor.matmul(ps, w_sb[:, k, :], xsb[:, k, :],
                                     start=(k == 0), stop=(k == KT - 1))
                t1 = rope.tile([128, 512], F32, tag="t1")
                t2 = rope.tile([128, 512], F32, tag="t2")
                t2sw = rope.tile([128, 512], F32, tag="t2sw")
                nc.vector.tensor_tensor(t1, ps, cc_sb, mybir.AluOpType.mult)
                nc.vector.tensor_tensor(t2, ps, ss_sb, mybir.AluOpType.mult)
                for b0 in (0, 64):
                    nc.sync.dma_start(t2sw[b0:b0 + 32], t2[b0 + 32:b0 + 64])
                    nc.sync.dma_start(t2sw[b0 + 32:b0 + 64], t2[b0:b0 + 32])
                nc.vector.tensor_tensor(dstT[:, ch, :], t1, t2sw,
                                        mybir.AluOpType.add)

            def v_proj(ch, tiles, half):
                """V for token blocks 2*half, 2*half+1 of chunk ch."""
                xsb, _, _ = tiles
                for tt in (2 * half, 2 * half + 1):
                    psv = psF.tile([128, 128], F32, tag="f", name="psv")
                    for k in range(KT):
                        nc.tensor.matmul(psv, xsb[:, k, tt * 128:(tt + 1) * 128],
                                         wv_sb[:, k, :],
                                         start=(k == 0), stop=(k == KT - 1))
                    ti = ch * 4 + tt
                    # one strided copy fills both heads' v slices
                    dst = vAB_sb[:, ti, :].rearrange(
                        "p (g c) -> p g c", g=2)[:, :, 0:64]
                    src = psv[:, :].rearrange("p (g c) -> p g c", g=2)
                    nc.vector.tensor_copy(dst, src)

            # ---------- phase-3 unit (output projection of one chunk) -------
            def out_proj(ch, cck4):
                """4 of the 8 wo k-tiles for chunk ch."""
                for cck in range(cck4 * 4, cck4 * 4 + 4):
                    psy = psF.tile([128, 512], F32, tag="f", name="psy")
                    nc.tensor.matmul(psy, wo_sb[:, cck, :], attnT_sb[:, ch, :],
                                     start=True, stop=True)
                    ysb = ph3.tile([128, 512], BF16, tag="ysb")
                    nc.vector.tensor_copy(ysb, psy)
                    # y writeback rides the idle GpSimd SWDGE queue so the
                    # Sync queue stays clear for latency-critical loads; the
                    # last chunks go back to Sync (idle by then, and SWDGE's
                    # ~1us/DMA issue overhead would stretch the tail)
                    eng = nc.sync if ch >= NCHUNK - 2 else nc.gpsimd
                    eng.dma_start(
                        y_d[cck * 128:(cck + 1) * 128,
                            ch * 512:(ch + 1) * 512], ysb)

            # ---------- filler queue (kept sorted by due, FIFO on ties) ----
            queue = []
            _seq = [0]

            def enqueue(emit, due):
                import bisect
                _seq[0] += 1
                bisect.insort(queue, (due, _seq[0], _Unit(emit, due)))

            def pop_due(tick):
                while queue and queue[0][0] <= tick:
                    queue.pop(0)[2].emit()

            def pop_budget():
                if queue:
                    queue.pop(0)[2].emit()

            def enqueue_phase1(bn):
                """QKV units for all 4 chunks of batch bn.  kT / vA / vB / xsb
                feed matmul STATIONARY operands, and the PE's LDWEIGHTS
                pull-ahead reorder (64-deep window) does NOT respect the sems
                that gate the paired matmul — so all of load/k/v must be
                emitted >=2 blocks (>=64 PE instructions) before the first
                consuming block (bn, 0).  q feeds only MOVING operands (sem-
                gated properly), so q units may land as late as their block."""
                g0 = bn * 4 * 16        # tick of block (bn, 0), 16 ticks/block
                kv_due = g0 - 32
                tiles = {}
                # All 4 loads first, then all k, then all v, then all q:
                # v_proj consumes xsb as a matmul STATIONARY, and the PE's
                # LDWEIGHTS pull-ahead can read it up to ~64 instructions
                # early -- the k units (32 matmuls) in between guarantee the
                # DMA has landed before any v LDWEIGHTS can issue.
                for c4 in range(4):
                    ch = bn * 4 + c4

                    def em_load(ch=ch):
                        tiles[ch] = load_x(ch)
                    enqueue(em_load, kv_due + c4)
                # dues are SPREAD across the window (not bunched in 4
                # ticks): pop_due otherwise dumps ~26us of PE work in one
                # go ahead of the scores, starving the ScalarE.  The band
                # order loads < k < v < q is preserved.
                for c4 in range(4):
                    ch = bn * 4 + c4
                    enqueue(
                        lambda ch=ch: qk_proj(ch, tiles[ch], wk_sb, kT_sb),
                        kv_due + 4 + c4)
                for c4 in range(4):
                    ch = bn * 4 + c4
                    for half in (0, 1):
                        enqueue(
                            lambda ch=ch, half=half: v_proj(ch, tiles[ch], half),
                            kv_due + 8 + 2 * c4 + half)
                # q last: frees this batch's xsb slots before the NEXT
                # batch's loads (sorted-queue order); q feeds only moving
                # operands, so closeness to its consumers is safe.
                for c4 in range(4):
                    ch = bn * 4 + c4
                    enqueue(
                        lambda ch=ch: qk_proj(ch, tiles[ch], wq_sb, qT_sb),
                        kv_due + 16 + 2 * c4)

            def enqueue_phase3(ch):
                for cck4 in (0, 1):
                    enqueue(lambda ch=ch, cck4=cck4: out_proj(ch, cck4), 1 << 30)

            # ---------- phase-2 block --------------------------------------
            def phase2_block(b, iq):
                g = (b * 4 + iq) * 16
                ch_i = b * 4 + iq
                posA = psA.tile([65, 512], F32, tag="pos", name="posA")
                posB = psA.tile([65, 512], F32, tag="pos", name="posB")
                pss = {}

                def scores(jb):
                    t = psS.tile([128, 1024], F32, tag="sc", name=f"sc{jb % 2}")
                    pss[jb] = t
                    ch_j = b * 4 + jb // 4
                    off_j = (jb % 4) * 128
                    for hh, b0 in ((0, 0), (1, 64)):
                        nc.tensor.matmul(
                            t[:, hh * 512:(hh + 1) * 512],
                            kT_sb[b0:b0 + 64, ch_j, off_j:off_j + 128],
                            qT_sb[b0:b0 + 64, ch_i, :],
                            start=True, stop=True, tile_position=(b0, 0))

                pop_due(g - 1)
                scores(0)
                scores(1)
                for jb in range(16):
                    pop_due(g + jb)
                    if jb == 1 and pending_fin:
                        # must emit before any pop_budget() can emit an
                        # out_proj that reads the attnT these fill in
                        for f in pending_fin:
                            f()
                        pending_fin.clear()
                    bias = (mb_sb[:, b * 16 + jb:b * 16 + jb + 1]
                            if use_mask else 0.0)
                    pT = ptp.tile([128, 1024], BF16, tag="pT", name="pT")
                    ptile = pss.pop(jb)
                    nc.scalar.activation(pT, ptile,
                                         mybir.ActivationFunctionType.Exp,
                                         bias=bias, scale=0.125)
                    if DEBUG_DUMP and b == 0 and iq == 0:
                        dsc = norm.tile([128, 1024], F32, tag="dbgsc")
                        nc.vector.tensor_copy(dsc, ptile)
                        nc.sync.dma_start(dbg_sc[:, jb, :], dsc)
                        nc.sync.dma_start(dbg_pt[:, jb, :], pT)
                    if jb + 2 < 16:
                        scores(jb + 2)
                    if jb % 2 == 1:
                        pop_budget()
                        if b == B - 1:
                            # no next-batch phase-1 exists; drain the
                            # out_proj backlog instead of leaving a tail
                            pop_budget()
                    for hh, pos in ((0, posA), (1, posB)):
                        nc.tensor.matmul(
                            pos,
                            vAB_sb[:, b * 16 + jb, hh * 65:(hh + 1) * 65],
                            pT[:, hh * 512:(hh + 1) * 512],
                            start=(jb == 0), stop=(jb == 15))
                # normalize: out = attn_out / colsum, written transposed
                # bf16.  ocp+csrow copies happen now (frees the pos banks);
                # the reciprocal, PE broadcast and multiply are deferred
                # into the next block so the PE never head-of-line waits on
                # the DVE.
                rows = []
                for hh, pos in ((0, posA), (1, posB)):
                    ocp = norm.tile([65, 512], F32, tag="ocp")
                    nc.vector.tensor_copy(ocp, pos)
                    # reciprocal_approx_fast needs a base-partition-0 input
                    # tile; feeding it a [64:65] slice mis-executes
                    csrow = norm.tile([1, 512], F32, tag="csrow")
                    nc.vector.tensor_copy(csrow, ocp[64:65, :])
                    rows.append((hh, ocp, csrow))

                def finish_norm():
                    for hh, ocp, csrow in rows:
                        csrec = norm.tile([1, 512], F32, tag="csrec")
                        nc.vector.reciprocal_approx_fast(out=csrec, in_=csrow)
                        # bf16 copy keeps the broadcast matmul out of fp32
                        # mode; the ~0.4% recip error scales whole columns
                        # uniformly and is negligible vs the 2e-2 budget
                        csrecb = norm.tile([1, 512], BF16, tag="csrecb")
                        nc.vector.tensor_copy(csrecb, csrec)
                        # broadcast recip to 64 partitions with a K=1 matmul
                        # (ones stationary x csrec moving)
                        csrep = psF.tile([64, 512], F32, tag="f",
                                         name="csrep")
                        nc.tensor.matmul(csrep, ones65[0:1, :], csrecb,
                                         start=True, stop=True)
                        if DEBUG_DUMP:
                            nc.sync.dma_start(
                                dbg_ocp[hh * 65:hh * 65 + 65, ch_i, :], ocp)
                        nc.vector.tensor_tensor(
                            attnT_sb[hh * 64:hh * 64 + 64, ch_i, :],
                            ocp[0:64, :], csrep, mybir.AluOpType.mult)
                if b == B - 1 and iq == 3:
                    finish_norm()   # last block: no next block to defer into
                else:
                    pending_fin.append(finish_norm)

            # ---------- program --------------------------------------------
            # prologue: all of batch 0's load/k/v inline (stationary-feeding
            # data must be far upstream of its consumers, see enqueue_phase1)
            t0 = {ch: load_x(ch, prologue=True) for ch in range(4)}
            for ch in range(4):
                qk_proj(ch, t0[ch], wk_sb, kT_sb)
            for ch in range(4):
                v_proj(ch, t0[ch], 0)
                v_proj(ch, t0[ch], 1)
            # only chunk 0's q is needed before block (0,0) starts (scores
            # consume qT as a sem-gated moving operand, so tight is safe);
            # q1-3 weave into the first blocks as filler
            qk_proj(0, t0[0], wq_sb, qT_sb)
            for ch in (1, 2, 3):
                enqueue(
                    lambda ch=ch: qk_proj(ch, t0[ch], wq_sb, qT_sb), ch - 1)

            pending_fin = []
            for b in range(B):
                if b + 1 < B:
                    enqueue_phase1(b + 1)
                for iq in range(4):
                    phase2_block(b, iq)
                    enqueue_phase3(b * 4 + iq)
            for f in pending_fin:
                f()
            pending_fin.clear()
            while queue:
                queue.pop(0)[2].emit()

    return nc


def _host_prep(x, key_padding_mask, dt, wq, wk, wv, wo):
    """Shared + per-core input arrays (all numpy)."""
    xT = np.ascontiguousarray(x.reshape(T, E).T).astype(ml_dtypes.bfloat16)

    # RoPE trig tables, rows [c;c;c;c] and [s;-s;s;-s] over 32-row blocks
    inv_freq = (1.0 / (THETA ** (np.arange(0, D, 2, dtype=np.float32) / D)))
    ang = dt.reshape(T).astype(np.float32)[None, :] * inv_freq[:, None]  # [32, T]
    cos = np.cos(ang).astype(np.float32)
    sin = np.sin(ang).astype(np.float32)
    cc = np.concatenate([cos, cos, cos, cos], axis=0)
    ssm = np.concatenate([sin, -sin, sin, -sin], axis=0)

    use_mask = bool(key_padding_mask.any())
    mb = None
    if use_mask:
        bias = np.where(key_padding_mask.reshape(T), NEG_INF, 0.0).astype(np.float32)
        # [128 j-in-block, B*16 block index]
        mb = np.ascontiguousarray(bias.reshape(B * 16, 128).T)

    # per-head channel permutation: [2r] then [2r+1] -> [r | 32+r]
    perm1 = np.concatenate([np.arange(0, D, 2), np.arange(1, D, 2)])

    per_core = []
    for c in range(NCORES):
        rows = []
        for h in range(c * HPC, (c + 1) * HPC):
            rows.append(h * D + perm1)
        rows = np.concatenate(rows)                      # permuted q/k rows
        rows_v = np.arange(c * CPC, (c + 1) * CPC)       # natural v rows
        # note: the 1/sqrt(D)=0.125 score scale is applied as the exp
        # activation's scale argument on device, not here
        wqT = np.ascontiguousarray(wq[rows].T).astype(ml_dtypes.bfloat16)
        wkT = np.ascontiguousarray(wk[rows].T).astype(ml_dtypes.bfloat16)
        wvT = np.ascontiguousarray(wv[rows_v].T).astype(ml_dtypes.bfloat16)
        woT = np.ascontiguousarray(wo[:, rows_v].T).astype(ml_dtypes.bfloat16)
        m = {"xT": xT, "cc": cc, "ss": ssm,
             "wqT": wqT, "wkT": wkT, "wvT": wvT, "woT": woT}
        if use_mask:
            m["mb"] = mb
        per_core.append(m)
    return per_core, use_mask


def kernel(x, key_padding_mask, dt, wq, wk, wv, wo, bo, _return_results=False):
    x = np.asarray(x, dtype=np.float32)
    key_padding_mask = np.asarray(key_padding_mask)
    dt = np.asarray(dt, dtype=np.float32)
    wq = np.asarray(wq, dtype=np.float32)
    wk = np.asarray(wk, dtype=np.float32)
    wv = np.asarray(wv, dtype=np.float32)
    wo = np.asarray(wo, dtype=np.float32)
    bo = np.asarray(bo, dtype=np.float32)

    in_maps, use_mask = _host_prep(x, key_padding_mask, dt, wq, wk, wv, wo)

    key = use_mask
    if key not in _prog_cache:
        prog = _build_program(use_mask)
        prog.finalize()
        _prog_cache[key] = prog
    nc = _prog_cache[key]

    res = run_bass_kernel_spmd(nc, in_maps, list(range(NCORES)))

    y = np.zeros((E, T), dtype=np.float32)
    for r in res.results:
        y += r["yT"].astype(np.float32)
    out = (y.T + bo[None, :]).reshape(B, S, E).astype(np.float32)
    if _return_results:
        return out, res
    return out

